# revision 1
# baseline (speedup 1.0000x reference)
"""Trainium2 Bass kernel: sparse (sliding-window) attention block, v3.

Full module per reference:
  RMSNorm -> fused QKV (5120x2880) -> YaRN RoPE -> GQA sliding-window(128)
  causal attention with learned sink logit -> out projection (2880x4096).

Sharding: tensor-parallel over heads across 8 cores. Core c owns q-heads
[8c, 8c+8) and kv-head c. Each core emits a partial [1024, 2880] bf16
output; the host sums the partials (f64) and adds out_b.

v3 = v1 attention backend (q-on-partitions AV with ones-column
denominators, tiny reciprocals, PE transposes for the out-proj lhsT)
plus the v2 scheduling wins:
  - both activation tables (Ln, Exp) prefetched with dummy ops at t0; no
    Square activation anywhere (x^2 via vector bf16 muls) -> no table
    swaps, no startup stall.
  - half-granularity pipeline: kv+q projection and rope for tokens
    [0,512) are followed immediately by attention+out-proj of tiles 0-3
    while the second half's QKV matmuls still run.
  - sm_scale folded into the Exp activation scale -> q and k share one
    plain cos/sin table pair (half the table DMA/SBUF).
  - masks multiplicative {0,1} bf16 applied to the exp'd probabilities.
  - y written bf16 in [128, 960] chunks DMA'd from the scalar queue.
"""

import math
import sys

import numpy as np

try:
    import concourse.bass as bass
except ImportError:  # pragma: no cover
    sys.path.insert(0, "/opt/trn_rl_repo")
    import concourse.bass as bass

import concourse.bacc as bacc
import concourse.tile as tile
from concourse import mybir
from concourse.masks import make_identity
from concourse.bass_utils import run_bass_kernel_spmd

import ml_dtypes

BF16 = ml_dtypes.bfloat16

T = 1024
HIDDEN = 2880
HD = 64
NH = 64
NKV = 8
SW = 128
NCORES = 8
HPC = NH // NCORES          # q heads per core = 8
QKV_DIM = HD * (NH + 2 * NKV)
SM_SCALE = 1.0 / math.sqrt(HD)

P = 128
KT = (HIDDEN + P - 1) // P   # 23 k-tiles over hidden (zero-padded to 2944)
KPAD = KT * P
NT = 5                       # qkv n-tiles of 128 (4 q-tiles + 1 kv-tile)
MT = T // P                  # 8 token tiles
AW = HD + 1                  # AV width: 64 v dims + denominator column

dt = mybir.dt

_CACHE = {}


# ----------------------------------------------------------------------------
# host-side helpers
# ----------------------------------------------------------------------------

def _rope_cos_sin(num_tokens):
    base = 150000.0
    scaling = 32.0
    init_ctx = 4096.0
    ntk_alpha = 1.0
    ntk_beta = 32.0
    d_half = HD / 2
    freq = base ** (np.arange(0, HD, 2, dtype=np.float32) / HD)
    concentration = 0.1 * math.log(scaling) + 1.0
    low = d_half * math.log(init_ctx / (ntk_beta * 2 * math.pi)) / math.log(base)
    high = d_half * math.log(init_ctx / (ntk_alpha * 2 * math.pi)) / math.log(base)
    interpolation = 1.0 / (scaling * freq)
    extrapolation = 1.0 / freq
    ramp = (np.arange(int(d_half), dtype=np.float32) - low) / (high - low)
    m = 1.0 - np.clip(ramp, 0.0, 1.0)
    inv_freq = interpolation * (1.0 - m) + extrapolation * m
    t = np.arange(num_tokens, dtype=np.float32)
    freqs = t[:, None] * inv_freq[None, :]
    cos = (np.cos(freqs) * concentration).astype(np.float32)
    sin = (np.sin(freqs) * concentration).astype(np.float32)
    return cos, sin  # [T, 32]


def _host_tables():
    """Plain (unscaled) replicated rope tables with the swap sign folded
    into sin: rope(u)[p] = u[p]*cos[p] + u[p^32]*sin_alt[p]."""
    cos, sin = _rope_cos_sin(T)  # [1024, 32]
    sgn = np.repeat([-1.0, 1.0], 32)[:, None].astype(np.float32)
    sgn = np.tile(sgn, (2, 1))  # [128, 1]
    cos_t = np.tile(cos.T, (4, 1)).astype(np.float32)          # [128, 1024]
    sin_t = (np.tile(sin.T, (4, 1)) * sgn).astype(np.float32)  # [128, 1024]
    return cos_t, sin_t


def _host_masks01():
    j = np.arange(P)[:, None]   # kt row (partition)
    i = np.arange(P)[None, :]   # q col (free)
    mask_prev = (j > i).astype(np.float32)    # dist in [1,127]
    mask_self = (j <= i).astype(np.float32)   # dist in [0,127]
    return np.concatenate([mask_prev, mask_self], axis=1).astype(BF16)


def _prep_core_inputs(core, x, norm_scale, qkv_w, qkv_b, out_w, sinks):
    q_end = NH * HD
    k_end = q_end + NKV * HD

    qrows = np.arange(core * HPC * HD, (core + 1) * HPC * HD)
    krows = np.arange(q_end + core * HD, q_end + (core + 1) * HD)
    vrows = np.arange(k_end + core * HD, k_end + (core + 1) * HD)
    # kv n-tile: v in partitions 0:64, k in 64:128
    rows = np.concatenate([qrows, vrows, krows])  # [640]

    wshard = (qkv_w[rows, :] * norm_scale[None, :]).astype(np.float32)
    bshard = qkv_b[rows].astype(np.float32)  # [640]

    # lhsT tiles: wq[n, kp, kt*128 + nc] = wshard[n*128 + nc, kt*128 + kp]
    wq = np.zeros((NT, P, KPAD), dtype=BF16)
    for n in range(NT):
        blk = wshard[n * P:(n + 1) * P, :]  # [128 n, 2880 k]
        for ki in range(KT):
            k0 = ki * P
            ksz = min(P, HIDDEN - k0)
            wq[n, :ksz, ki * P:ki * P + P] = blk[:, k0:k0 + ksz].T.astype(BF16)

    cols = np.arange(core * HPC * HD, (core + 1) * HPC * HD)
    wo = out_w[:, cols].T.astype(np.float32)  # [512 hd, 2880 H]
    wout = wo.reshape(4, P, HIDDEN).astype(BF16)

    bqkv = bshard.reshape(NT, P).T.copy().astype(np.float32)  # [128, 5]

    cos_t, sin_t = _host_tables()  # [128, 1024] f32 each

    xt = np.zeros((KPAD, T), dtype=BF16)
    xt[:HIDDEN] = x.T.astype(BF16)

    esink = np.exp(sinks[core * HPC:(core + 1) * HPC].astype(np.float64))
    esink = np.broadcast_to(esink.astype(np.float32), (P, HPC)).copy()

    return {
        "xt": xt,                               # [2944, 1024] bf16
        "wq": wq,                               # [5, 128, 2944] bf16
        "wout": wout,                           # [4, 128, 2880] bf16
        "bqkv": bqkv,                           # [128, 5] f32
        "cos_t": cos_t, "sin_t": sin_t,         # [128, 1024] f32
        "sin_neg": (-sin_t).copy(),             # sin_alt[p^32] = -sin_alt[p]
        "mask": _host_masks01(),                # [128, 256] bf16
        "esink": esink,                         # [128, 8] f32
    }


# ----------------------------------------------------------------------------
# device kernel (Tile)
# ----------------------------------------------------------------------------

def build_nc():
    nc = bacc.Bacc("TRN2", target_bir_lowering=False, debug=False)

    xt_d = nc.dram_tensor("xt", [KPAD, T], dt.bfloat16, kind="ExternalInput").ap()
    wq_d = nc.dram_tensor("wq", [NT, P, KPAD], dt.bfloat16, kind="ExternalInput").ap()
    wout_d = nc.dram_tensor("wout", [4, P, HIDDEN], dt.bfloat16, kind="ExternalInput").ap()
    bqkv_d = nc.dram_tensor("bqkv", [P, NT], dt.float32, kind="ExternalInput").ap()
    cos_d = nc.dram_tensor("cos_t", [P, T], dt.float32, kind="ExternalInput").ap()
    sin_d = nc.dram_tensor("sin_t", [P, T], dt.float32, kind="ExternalInput").ap()
    sneg_d = nc.dram_tensor("sin_neg", [P, T], dt.float32, kind="ExternalInput").ap()
    mask_d = nc.dram_tensor("mask", [P, 2 * P], dt.bfloat16, kind="ExternalInput").ap()
    esink_d = nc.dram_tensor("esink", [P, HPC], dt.float32, kind="ExternalInput").ap()
    y_d = nc.dram_tensor("y", [T, HIDDEN], dt.bfloat16, kind="ExternalOutput").ap()

    YC = 480

    def bcast_mid(ap2d, n):
        """[P, F] -> [P, n, F] with a 0-step middle dim (free broadcast)."""
        return bass.AP(tensor=ap2d.tensor, offset=ap2d.offset,
                       ap=[ap2d.ap[0], [0, n]] + list(ap2d.ap[1:]))

    with tile.TileContext(nc) as tc:
        with (
            tc.tile_pool(name="const", bufs=1) as const,
            tc.tile_pool(name="res", bufs=1) as res,
            tc.tile_pool(name="qkvp", bufs=2) as qkvp,
            tc.tile_pool(name="xsqp", bufs=2) as xsqp,
            tc.tile_pool(name="ropep", bufs=2) as ropep,
            tc.tile_pool(name="ptp", bufs=3) as ptp,
            tc.tile_pool(name="smallp", bufs=3) as smallp,
            tc.tile_pool(name="anp", bufs=3) as anp,
            tc.tile_pool(name="attp", bufs=3) as attp,
            tc.tile_pool(name="ysbp", bufs=3) as ysbp,
            tc.tile_pool(name="pA", bufs=2, space="PSUM") as pA,
            tc.tile_pool(name="pY", bufs=2, space="PSUM") as pY,
            tc.tile_pool(name="pS", bufs=2, space="PSUM") as pS,
            tc.tile_pool(name="pG", bufs=2, space="PSUM") as pG,
        ):
            # ---- constants ----
            zbias = const.tile([P, 1], dt.float32, tag="zbias", name="zbias")
            nc.vector.memset(zbias, 0.0)
            eps_t = const.tile([1, 1], dt.float32, tag="eps", name="eps_t")
            nc.vector.memset(eps_t, 1e-5)
            ones = const.tile([P, 1], dt.bfloat16, tag="ones", name="ones")
            nc.vector.memset(ones, 1.0)
            identb = const.tile([P, P], dt.bfloat16, tag="identb", name="identb")
            make_identity(nc, identb)
            # prefetch both activation tables (Ln, Exp) with dummy ops
            dmy = const.tile([1, 2], dt.float32, tag="dmy", name="dmy")
            nc.scalar.activation(dmy[:, 0:1], eps_t,
                                 mybir.ActivationFunctionType.Ln,
                                 bias=eps_t)
            nc.scalar.activation(dmy[:, 1:2], eps_t,
                                 mybir.ActivationFunctionType.Exp,
                                 bias=zbias[0:1, :])

            # ---- DMA issue (sync queue; y outputs go on the scalar queue) ----
            wq_sb = [res.tile([P, KPAD], dt.bfloat16, tag=f"wq{n}", name=f"wq{n}")
                     for n in range(NT)]
            xt_sb = res.tile([P, KT, T], dt.bfloat16, tag="xt", name="xt")
            wout_sb = [res.tile([P, HIDDEN], dt.bfloat16, tag=f"wout{kk}",
                                name=f"wout{kk}") for kk in range(4)]
            cos_sb = const.tile([P, T], dt.float32, tag="cos", name="cos_sb")
            sin_sb = const.tile([P, T], dt.float32, tag="sin", name="sin_sb")
            sneg_sb = const.tile([P, T], dt.float32, tag="sneg", name="sneg_sb")
            mask_sb = const.tile([P, 2 * P], dt.bfloat16, tag="mask", name="mask_sb")
            esink_sb = const.tile([P, HPC], dt.float32, tag="esink", name="esink_sb")
            bqkv_sb = const.tile([P, NT], dt.float32, tag="bqkv", name="bqkv_sb")

            HK = KPAD // 2
            HO = HIDDEN // 2
            HT = T // 2

            def dma(out, in_):
                nc.sync.dma_start(out=out, in_=in_)

            def dma2(out, in_):
                nc.scalar.dma_start(out=out, in_=in_)

            # xt first (rsq path is the long pole); first tile split for an
            # early pipeline start; weights interleaved in need order
            dma(xt_sb[:, 0, 0:256], xt_d[0:P, 0:256])
            dma2(xt_sb[:, 0, 256:512], xt_d[0:P, 256:512])
            dma(xt_sb[:, 0, 512:768], xt_d[0:P, 512:768])
            dma2(xt_sb[:, 0, 768:], xt_d[0:P, 768:])
            dma(wq_sb[4][:, :HK], wq_d[4, :, :HK])
            for ki in range(1, 5):
                (dma if ki % 2 else dma2)(xt_sb[:, ki, :],
                                          xt_d[ki * P:(ki + 1) * P, :])
            dma(wq_sb[4][:, HK:], wq_d[4, :, HK:])
            for ki in range(5, 9):
                (dma if ki % 2 else dma2)(xt_sb[:, ki, :],
                                          xt_d[ki * P:(ki + 1) * P, :])
            dma(wq_sb[0][:, :HK], wq_d[0, :, :HK])
            for ki in range(9, 13):
                (dma if ki % 2 else dma2)(xt_sb[:, ki, :],
                                          xt_d[ki * P:(ki + 1) * P, :])
            dma(wq_sb[0][:, HK:], wq_d[0, :, HK:])
            for ki in range(13, 18):
                (dma if ki % 2 else dma2)(xt_sb[:, ki, :],
                                          xt_d[ki * P:(ki + 1) * P, :])
            dma(cos_sb[:, :HT], cos_d[:, :HT])
            dma2(sin_sb[:, :HT], sin_d[:, :HT])
            dma(sneg_sb[:, :HT], sneg_d[:, :HT])
            for ki in range(18, KT):
                (dma if ki % 2 else dma2)(xt_sb[:, ki, :],
                                          xt_d[ki * P:(ki + 1) * P, :])
            dma(cos_sb[:, HT:], cos_d[:, HT:])
            dma2(sin_sb[:, HT:], sin_d[:, HT:])
            dma2(sneg_sb[:, HT:], sneg_d[:, HT:])
            dma(mask_sb, mask_d)
            dma(esink_sb, esink_d)
            dma(bqkv_sb, bqkv_d)
            dma(wq_sb[1][:, :HK], wq_d[1, :, :HK])
            dma(wq_sb[1][:, HK:], wq_d[1, :, HK:])
            dma(wout_sb[0][:, :HO], wout_d[0, :, :HO])
            dma(wout_sb[0][:, HO:], wout_d[0, :, HO:])
            dma(wq_sb[2][:, :HK], wq_d[2, :, :HK])
            dma(wq_sb[2][:, HK:], wq_d[2, :, HK:])
            for kk in range(1, 4):
                dma(wout_sb[kk][:, :HO], wout_d[kk, :, :HO])
                dma(wout_sb[kk][:, HO:], wout_d[kk, :, HO:])
            dma(wq_sb[3][:, :HK], wq_d[3, :, :HK])
            dma(wq_sb[3][:, HK:], wq_d[3, :, HK:])

            # ---- sum of squares: vector bf16 squares + PE ones-reduction ----
            psum_ssq = [pY.tile([1, 512], dt.float32, tag="py", name=f"ssq{h}")
                        for h in range(2)]  # holds 2 of pY's 3 bufs until rsq
            for ki in range(KT):
                xsq = xsqp.tile([P, T], dt.bfloat16, tag="xsq", name="xsq")
                if ki == 0:
                    for hh in range(2):
                        nc.vector.tensor_mul(xsq[:, hh * 512:hh * 512 + 512],
                                             xt_sb[:, 0, hh * 512:hh * 512 + 512],
                                             xt_sb[:, 0, hh * 512:hh * 512 + 512])
                else:
                    nc.vector.tensor_mul(xsq, xt_sb[:, ki, :], xt_sb[:, ki, :])
                for half in range(2):
                    nc.tensor.matmul(
                        psum_ssq[half],
                        ones,
                        xsq[:, half * 512:half * 512 + 512],
                        start=(ki == 0), stop=(ki == KT - 1),
                    )

            # rsq_b = exp(-0.5*ln(ssq/H + eps)) broadcast to 128 partitions
            lnm = res.tile([1, T], dt.float32, tag="lnm", name="lnm")
            for half in range(2):
                nc.scalar.activation(lnm[:, half * 512:half * 512 + 512],
                                     psum_ssq[half],
                                     mybir.ActivationFunctionType.Ln,
                                     bias=eps_t, scale=1.0 / HIDDEN)
            rinv = res.tile([1, T], dt.float32, tag="rinv", name="rinv")
            nc.scalar.activation(rinv, lnm, mybir.ActivationFunctionType.Exp,
                                 bias=zbias[0:1, :], scale=-0.5)
            rsq_b = res.tile([P, T], dt.float32, tag="rsq", name="rsq_b")
            nc.gpsimd.partition_broadcast(rsq_b, rinv)

            # ---- qkv projection (scaled+biased, bf16) ----
            def qkv_tile(n, half, dst):
                """dst <- (W x)*rsq + b for columns [half*512, ...+512)."""
                c0 = half * 512
                pq = pA.tile([P, 512], dt.float32, tag="pb", name="pb")
                for ki in range(KT):
                    nc.tensor.matmul(
                        pq,
                        wq_sb[n][:, ki * P:ki * P + P],
                        xt_sb[:, ki, c0:c0 + 512],
                        start=(ki == 0), stop=(ki == KT - 1),
                    )
                nc.vector.tensor_mul(dst, pq, rsq_b[:, c0:c0 + 512])
                nc.vector.tensor_scalar_add(dst, dst, bqkv_sb[:, n:n + 1])

            qra = res.tile([64, HPC, T], dt.bfloat16, tag="qra", name="qra")
            krope = res.tile([64, T], dt.bfloat16, tag="krope", name="krope")
            # all 8 token-major v tiles in one buffer; ones column prefilled
            vtok = res.tile([P, MT, AW], dt.bfloat16, tag="vtok", name="vtok")
            nc.vector.memset(vtok[:, :, HD:HD + 1], 1.0)

            qkvT4 = res.tile([P, T], dt.bfloat16, tag="qkvT4", name="qkvT4")

            def kv_phase(half):
                c0 = half * 512
                qkv_tile(4, half, qkvT4[:, c0:c0 + 512])
                for j in range(4):
                    b = half * 4 + j
                    pv = pG.tile([P, HD], dt.bfloat16, tag="pg", name="pv")
                    nc.tensor.transpose(pv, qkvT4[0:64, b * P:(b + 1) * P],
                                        identb[:64, :64])
                    nc.vector.tensor_copy(vtok[:, b, 0:HD], pv)
                # k rope (rows 64:128) at half width; the swapped-operand term
                # is computed with partition-offset muls (no copies)
                ktc = ropep.tile([P, 512], dt.bfloat16, tag="htc", name="ktc")
                nc.vector.tensor_mul(ktc[64:128, :], qkvT4[64:128, c0:c0 + 512],
                                     cos_sb[64:128, c0:c0 + 512])
                kts = ropep.tile([P, 512], dt.bfloat16, tag="hts", name="kts")
                for a in (64, 96):
                    b_ = a ^ 32
                    nc.gpsimd.tensor_mul(kts[a:a + 32, :],
                                         qkvT4[b_:b_ + 32, c0:c0 + 512],
                                         sneg_sb[b_:b_ + 32, c0:c0 + 512])
                nc.vector.tensor_add(krope[:, c0:c0 + 512], ktc[64:128, :],
                                     kts[64:128, :])

            def q_phase(half):
                c0 = half * 512
                for n in range(4):
                    qkvt = qkvp.tile([P, 512], dt.bfloat16, tag="qkvT",
                                     name=f"qkvt{n}")
                    qkv_tile(n, half, qkvt)
                    # rope at half width; swapped term via partition-offset muls
                    tc_ = ropep.tile([P, 512], dt.bfloat16, tag="htc", name="htc")
                    nc.vector.tensor_mul(tc_, qkvt, cos_sb[:, c0:c0 + 512])
                    ts_ = ropep.tile([P, 512], dt.bfloat16, tag="hts", name="hts")
                    for a in range(0, P, 32):
                        b_ = a ^ 32
                        nc.gpsimd.tensor_mul(ts_[a:a + 32, :],
                                             qkvt[b_:b_ + 32, :],
                                             sneg_sb[b_:b_ + 32, c0:c0 + 512])
                    for i in range(2):
                        b0 = 64 * i
                        nc.vector.tensor_add(qra[:, 2 * n + i, c0:c0 + 512],
                                             tc_[b0:b0 + 64, :],
                                             ts_[b0:b0 + 64, :])

            # ---- attention + out-projection for one token tile ----
            def attention(b):
                pt = ptp.tile([P, 2, HPC, P], dt.bfloat16, tag="pt", name="pt")
                kts = [(0, b - 1), (1, b)] if b > 0 else [(1, b)]
                # per-group pipeline: group 0 (heads 0-3, q-tiles 0/1) runs
                # fully while the later q-tiles' rope may still be in flight
                rec8 = smallp.tile([P, HPC], dt.float32, tag="rec8", name="rec8")
                an = anp.tile([P, HPC, HD], dt.bfloat16, tag="an", name="an")
                att = attp.tile([P, 4, P], dt.bfloat16, tag="att", name="att")
                a2 = an.rearrange("p a b -> p (a b)")
                for g in range(2):
                    g0 = 4 * g
                    for s, kt in kts:
                        ps = pS.tile([P, 4, P], dt.float32, tag="sc", name="sc")
                        nc.tensor.matmul(
                            ps,
                            krope[:, kt * P:(kt + 1) * P],
                            qra[:, g0:g0 + 4, b * P:(b + 1) * P],
                            start=True, stop=True)
                        nc.scalar.activation(pt[:, s, g0:g0 + 4, :], ps,
                                             mybir.ActivationFunctionType.Exp,
                                             bias=zbias, scale=SM_SCALE)
                        m0 = 0 if s == 0 else P
                        nc.gpsimd.tensor_mul(pt[:, s, g0:g0 + 4, :],
                                              pt[:, s, g0:g0 + 4, :],
                                              bcast_mid(mask_sb[:, m0:m0 + P], 4))
                    # AV with ones column -> denominators in column 64
                    pg = pG.tile([P, 4, AW], dt.float32, tag="pg", name="pg")
                    for j in range(4):
                        h = g0 + j
                        for idx, (s, kt) in enumerate(kts):
                            nc.tensor.matmul(pg[:, j, :], pt[:, s, h, :],
                                             vtok[:, kt, :],
                                             start=(idx == 0),
                                             stop=(idx == len(kts) - 1))
                    nc.vector.tensor_add(rec8[:, g0:g0 + 4],
                                         pg[:, :, HD:HD + 1],
                                         esink_sb[:, g0:g0 + 4])
                    nc.vector.reciprocal(rec8[:, g0:g0 + 4], rec8[:, g0:g0 + 4])
                    rec3 = bass.AP(tensor=rec8.tensor,
                                   offset=rec8[:, g0:g0 + 4].offset,
                                   ap=[rec8.ap[0], [1, 4], [0, HD]])
                    nc.vector.tensor_mul(an[:, g0:g0 + 4, :], pg[:, :, 0:HD],
                                         rec3)
                    # transpose to out-proj lhsT layout [128 hd, 128 tok]
                    for jj in range(2):
                        kk = 2 * g + jj
                        pat = pG.tile([P, P], dt.bfloat16, tag="pg", name="pat")
                        nc.tensor.transpose(pat, a2[:, kk * P:(kk + 1) * P],
                                            identb)
                        if kk % 2 == 0:
                            nc.vector.tensor_copy(att[:, kk, :], pat)
                        else:
                            nc.scalar.copy(att[:, kk, :], pat)

                # out projection, bf16 out; y DMAs ride the scalar queue
                # (the last tile is drained in 480-chunks on both queues to
                # shorten the kernel tail)
                for hy in range(3):
                    ysb = ysbp.tile([P, 2, YC], dt.bfloat16, tag="ysb", name="ysb")
                    for j in range(2):
                        ch = 2 * hy + j
                        o0 = ch * YC
                        pyt = pY.tile([P, 512], dt.float32, tag="py", name="py")
                        for kk in range(4):
                            nc.tensor.matmul(pyt[:, 0:YC], att[:, kk, :],
                                             wout_sb[kk][:, o0:o0 + YC],
                                             start=(kk == 0), stop=(kk == 3))
                        if ch % 2 == 0:
                            nc.vector.tensor_copy(ysb[:, j, :], pyt[:, 0:YC])
                        else:
                            nc.scalar.copy(ysb[:, j, :], pyt[:, 0:YC])
                        if b == MT - 1:
                            for q in range(2):
                                eng = nc.sync if q == 0 else nc.scalar
                                q0 = o0 + q * (YC // 2)
                                eng.dma_start(
                                    out=y_d[b * P:(b + 1) * P, q0:q0 + YC // 2],
                                    in_=ysb[:, j, q * (YC // 2):(q + 1) * (YC // 2)])
                    if b < MT - 1:
                        nc.sync.dma_start(
                            out=y_d[b * P:(b + 1) * P,
                                    hy * 2 * YC:(hy + 1) * 2 * YC],
                            in_=ysb)

            kv_phase(0)
            q_phase(0)
            attention(0)
            attention(1)
            attention(2)
            kv_phase(1)
            attention(3)
            q_phase(1)
            for j in range(4):
                attention(4 + j)

    nc.compile()
    return nc


# ----------------------------------------------------------------------------
# public entry
# ----------------------------------------------------------------------------

LAST_RESULTS = None


def kernel(x, norm_scale, qkv_w, qkv_b, out_w, out_b, sinks):
    global LAST_RESULTS
    x = np.asarray(x, dtype=np.float32)
    norm_scale = np.asarray(norm_scale, dtype=np.float32)
    qkv_w = np.asarray(qkv_w, dtype=np.float32)
    qkv_b = np.asarray(qkv_b, dtype=np.float32)
    out_w = np.asarray(out_w, dtype=np.float32)
    out_b = np.asarray(out_b, dtype=np.float32)
    sinks = np.asarray(sinks, dtype=np.float32)

    if "nc" not in _CACHE:
        _CACHE["nc"] = build_nc()
    nc = _CACHE["nc"]

    in_maps = [
        _prep_core_inputs(c, x, norm_scale, qkv_w, qkv_b, out_w, sinks)
        for c in range(NCORES)
    ]
    import os
    tmpdir = os.environ.get("BASS_TMPDIR") or None
    res = run_bass_kernel_spmd(nc, in_maps, core_ids=list(range(NCORES)),
                               tmpdir=tmpdir)
    LAST_RESULTS = res
    y = np.zeros((T, HIDDEN), dtype=np.float64)
    for c in range(NCORES):
        y += res.results[c]["y"].astype(np.float64)
    y += out_b.astype(np.float64)[None, :]
    return y.astype(np.float32)



# revision 13
# speedup vs baseline: 1.1496x; 1.1496x over previous
"""Trainium2 Bass kernel: sparse (sliding-window) attention block, v4.

Full module per reference:
  RMSNorm -> fused QKV (5120x2880) -> YaRN RoPE -> GQA sliding-window(128)
  causal attention with learned sink logit -> out projection (2880x4096).

Sharding: tensor-parallel over heads across 8 cores. Core c owns q-heads
[8c, 8c+8) and kv-head c. Each core emits a partial [1024, 2880] bf16
output; the host sums the partials (f64) and adds out_b.

v4 changes over v3 (all bf16 GEMMs kept -- fp8 matmuls were measured to
add ~8e-2 rel error through the logits, far over budget):
  - sum-of-squares via host-precomputed x^2 in fp8e4 + PE DoubleRow
    ones-reduction (2.6us PE instead of 9.8us PE + 10.8us DVE).
  - qkv bias folded into the matmul: xt row 2880 holds rms = 1/rsq
    (written on device), wq column k=2880 holds qkv_b, so
    psum = W x + b/rsq and one DVE mul by rsq finishes the tile.
  - rope tables in bf16 (DVE 2x mode) and indexed at the destination
    partition, so no separate negated-sin table is needed.
  - all rope sign-muls and mask muls on DVE (gpsimd ops measured
    1.1-1.7us each and serialized the attention pipeline in v3).
  - attention split into attn_front(b) (scores..att tile) and
    out_proj(b); out_proj(b-1) is emitted after attn_front(b) so its
    24 matmuls fill the exp/mask latency gaps on the PE.
"""

import math
import sys

import numpy as np

try:
    import concourse.bass as bass
except ImportError:  # pragma: no cover
    sys.path.insert(0, "/opt/trn_rl_repo")
    import concourse.bass as bass

import concourse.bacc as bacc
import concourse.tile as tile
from concourse import mybir
from concourse.masks import make_identity
from concourse.bass_utils import run_bass_kernel_spmd

import ml_dtypes

BF16 = ml_dtypes.bfloat16
F8 = ml_dtypes.float8_e4m3

T = 1024
HIDDEN = 2880
HD = 64
NH = 64
NKV = 8
SW = 128
NCORES = 8
HPC = NH // NCORES          # q heads per core = 8
QKV_DIM = HD * (NH + 2 * NKV)
SM_SCALE = 1.0 / math.sqrt(HD)

P = 128
KT = (HIDDEN + P - 1) // P   # 23 k-tiles over hidden (zero-padded to 2944)
KPAD = KT * P
KSQ = 24                     # xsq k-tiles (padded to an even count for DoubleRow)
NT = 5                       # qkv n-tiles of 128 (4 q-tiles + 1 kv-tile)
MT = T // P                  # 8 token tiles
AW = HD + 1                  # AV width: 64 v dims + denominator column
BIAS_K = HIDDEN              # contraction index carrying the bias (ki=22, kp=64)

dt = mybir.dt

_CACHE = {}


# ----------------------------------------------------------------------------
# host-side helpers
# ----------------------------------------------------------------------------

def _rope_cos_sin(num_tokens):
    base = 150000.0
    scaling = 32.0
    init_ctx = 4096.0
    ntk_alpha = 1.0
    ntk_beta = 32.0
    d_half = HD / 2
    freq = base ** (np.arange(0, HD, 2, dtype=np.float32) / HD)
    concentration = 0.1 * math.log(scaling) + 1.0
    low = d_half * math.log(init_ctx / (ntk_beta * 2 * math.pi)) / math.log(base)
    high = d_half * math.log(init_ctx / (ntk_alpha * 2 * math.pi)) / math.log(base)
    interpolation = 1.0 / (scaling * freq)
    extrapolation = 1.0 / freq
    ramp = (np.arange(int(d_half), dtype=np.float32) - low) / (high - low)
    m = 1.0 - np.clip(ramp, 0.0, 1.0)
    inv_freq = interpolation * (1.0 - m) + extrapolation * m
    t = np.arange(num_tokens, dtype=np.float32)
    freqs = t[:, None] * inv_freq[None, :]
    cos = (np.cos(freqs) * concentration).astype(np.float32)
    sin = (np.sin(freqs) * concentration).astype(np.float32)
    return cos, sin  # [T, 32]


def _host_tables():
    """Replicated rope tables, bf16, indexed at the DESTINATION partition:
    rope(u)[p] = u[p]*cos_t[p] + u[p^32]*sin_t[p]."""
    cos, sin = _rope_cos_sin(T)  # [1024, 32]
    sgn = np.repeat([-1.0, 1.0], 32)[:, None].astype(np.float32)
    sgn = np.tile(sgn, (2, 1))  # [128, 1]
    cos_t = np.tile(cos.T, (4, 1)).astype(BF16)          # [128, 1024]
    sin_t = (np.tile(sin.T, (4, 1)) * sgn).astype(BF16)  # [128, 1024]
    # walrus requires both SBUF inputs of a TensorTensor op to share the
    # base partition, so the sin table is indexed at the SOURCE partition:
    # ts[p] = u[p^32]*sin_t[p] = u[p^32]*sneg_t[p^32], sneg_t = -sin_t.
    return cos_t, (-sin_t.astype(np.float32)).astype(BF16)


def _host_masks01():
    j = np.arange(P)[:, None]   # kt row (partition)
    i = np.arange(P)[None, :]   # q col (free)
    mask_prev = (j > i).astype(np.float32)    # dist in [1,127]
    mask_self = (j <= i).astype(np.float32)   # dist in [0,127]
    return np.concatenate([mask_prev, mask_self], axis=1).astype(BF16)


def _prep_core_inputs(core, x, norm_scale, qkv_w, qkv_b, out_w, sinks):
    q_end = NH * HD
    k_end = q_end + NKV * HD

    qrows = np.arange(core * HPC * HD, (core + 1) * HPC * HD)
    krows = np.arange(q_end + core * HD, q_end + (core + 1) * HD)
    vrows = np.arange(k_end + core * HD, k_end + (core + 1) * HD)
    # kv n-tile: v in partitions 0:64, k in 64:128
    rows = np.concatenate([qrows, vrows, krows])  # [640]

    wshard = (qkv_w[rows, :] * norm_scale[None, :]).astype(np.float32)
    bshard = qkv_b[rows].astype(np.float32)  # [640]

    # lhsT tiles: wq[n, kp, ki*128 + nc] = wshard[n*128 + nc, ki*128 + kp];
    # the bias rides contraction index k=2880 (ki=22, kp=64) whose xt row is
    # filled with rms = 1/rsq on device.
    wq = np.zeros((NT, P, KPAD), dtype=BF16)
    for n in range(NT):
        blk = wshard[n * P:(n + 1) * P, :]  # [128 n, 2880 k]
        for ki in range(KT):
            k0 = ki * P
            ksz = min(P, HIDDEN - k0)
            wq[n, :ksz, ki * P:ki * P + P] = blk[:, k0:k0 + ksz].T.astype(BF16)
        wq[n, BIAS_K - 22 * P, 22 * P:23 * P] = bshard[n * P:(n + 1) * P].astype(BF16)

    cols = np.arange(core * HPC * HD, (core + 1) * HPC * HD)
    wo = out_w[:, cols].T.astype(np.float32)  # [512 hd, 2880 H]
    wout = wo.reshape(4, P, HIDDEN).astype(BF16)

    cos_t, sin_t = _host_tables()  # [128, 1024] bf16

    xt = np.zeros((KPAD, T), dtype=BF16)
    xt[:HIDDEN] = x.T.astype(BF16)

    xsq = np.zeros((KSQ * P, T), dtype=F8)
    xsq[:HIDDEN] = np.clip(x.T.astype(np.float64) ** 2, 0, 448).astype(F8)

    esink = np.exp(sinks[core * HPC:(core + 1) * HPC].astype(np.float64))
    esink = np.broadcast_to(esink.astype(np.float32), (P, HPC)).copy()

    return {
        "xt": xt,                               # [2944, 1024] bf16
        "xsq": xsq,                             # [3072, 1024] fp8e4
        "wq": wq,                               # [5, 128, 2944] bf16
        "wout": wout,                           # [4, 128, 2880] bf16
        "cos_t": cos_t, "sin_t": sin_t,         # [128, 1024] bf16
        "mask": _host_masks01(),                # [128, 256] bf16
        "esink": esink,                         # [128, 8] f32
    }


# ----------------------------------------------------------------------------
# device kernel (Tile)
# ----------------------------------------------------------------------------

def build_nc():
    nc = bacc.Bacc("TRN2", target_bir_lowering=False, debug=False)

    xt_d = nc.dram_tensor("xt", [KPAD, T], dt.bfloat16, kind="ExternalInput").ap()
    xsq_d = nc.dram_tensor("xsq", [KSQ * P, T], dt.float8e4,
                           kind="ExternalInput").ap()
    wq_d = nc.dram_tensor("wq", [NT, P, KPAD], dt.bfloat16, kind="ExternalInput").ap()
    wout_d = nc.dram_tensor("wout", [4, P, HIDDEN], dt.bfloat16,
                            kind="ExternalInput").ap()
    cos_d = nc.dram_tensor("cos_t", [P, T], dt.bfloat16, kind="ExternalInput").ap()
    sin_d = nc.dram_tensor("sin_t", [P, T], dt.bfloat16, kind="ExternalInput").ap()
    mask_d = nc.dram_tensor("mask", [P, 2 * P], dt.bfloat16, kind="ExternalInput").ap()
    esink_d = nc.dram_tensor("esink", [P, HPC], dt.float32, kind="ExternalInput").ap()
    y_d = nc.dram_tensor("y", [T, HIDDEN], dt.bfloat16, kind="ExternalOutput").ap()

    YC = 480

    def bcast_mid(ap2d, n):
        """[P, F] -> [P, n, F] with a 0-step middle dim (free broadcast)."""
        return bass.AP(tensor=ap2d.tensor, offset=ap2d.offset,
                       ap=[ap2d.ap[0], [0, n]] + list(ap2d.ap[1:]))

    def dram_tiles(d_ap, r0, nt, c0, csz):
        """[nt*128 rows from r0, cols c0:c0+csz] of a 2D dram tensor as a
        [128, nt, csz] AP (partition-major k-tiles)."""
        rstride = d_ap.ap[0][0]
        return bass.AP(tensor=d_ap.tensor, offset=d_ap.offset + r0 * rstride + c0,
                       ap=[[rstride, P], [P * rstride, nt], [1, csz]])

    with tile.TileContext(nc) as tc:
        with (
            tc.tile_pool(name="const", bufs=1) as const,
            tc.tile_pool(name="res", bufs=1) as res,
            tc.tile_pool(name="xsqp", bufs=2) as xsqp,
            tc.tile_pool(name="qkvp", bufs=2) as qkvp,
            tc.tile_pool(name="ropep", bufs=2) as ropep,
            tc.tile_pool(name="ptp", bufs=2) as ptp,
            tc.tile_pool(name="smallp", bufs=3) as smallp,
            tc.tile_pool(name="anp", bufs=2) as anp,
            tc.tile_pool(name="attp", bufs=2) as attp,
            tc.tile_pool(name="ysbp", bufs=3) as ysbp,
            tc.tile_pool(name="pA", bufs=2, space="PSUM") as pA,
            tc.tile_pool(name="pY", bufs=2, space="PSUM") as pY,
            tc.tile_pool(name="pS", bufs=2, space="PSUM") as pS,
            tc.tile_pool(name="pG", bufs=2, space="PSUM") as pG,
        ):
            # ---- constants ----
            zbias = const.tile([P, 1], dt.float32, tag="zbias", name="zbias")
            nc.vector.memset(zbias, 0.0)
            eps_t = const.tile([1, 1], dt.float32, tag="eps", name="eps_t")
            nc.vector.memset(eps_t, 1e-5)
            eps128 = const.tile([P, 1], dt.float32, tag="eps128", name="eps128")
            nc.vector.memset(eps128, 1e-5)
            # all-ones stationary [K, 2, 128]: every psum partition gets ssq,
            # so no partition_broadcast is needed for rsq.
            ones8 = const.tile([P, 2, P], dt.float8e4, tag="ones8", name="ones8")
            nc.vector.memset(ones8, 1.0)
            identb = const.tile([P, P], dt.bfloat16, tag="identb", name="identb")
            make_identity(nc, identb)
            # prefetch the Ln+Exp(+Copy) activation table set with dummy ops
            dmy = const.tile([1, 2], dt.float32, tag="dmy", name="dmy")
            nc.scalar.activation(dmy[:, 0:1], eps_t,
                                 mybir.ActivationFunctionType.Ln,
                                 bias=eps_t)
            nc.scalar.activation(dmy[:, 1:2], eps_t,
                                 mybir.ActivationFunctionType.Exp,
                                 bias=zbias[0:1, :])

            # ---- persistent SBUF ----
            wq_sb = [res.tile([P, KPAD], dt.bfloat16, tag=f"wq{n}", name=f"wq{n}")
                     for n in range(NT)]
            xt_sb = res.tile([P, KT, T], dt.bfloat16, tag="xt", name="xt")
            wout_sb = [res.tile([P, HIDDEN], dt.bfloat16, tag=f"wout{kk}",
                                name=f"wout{kk}") for kk in range(4)]
            cos_sb = const.tile([P, T], dt.bfloat16, tag="cos", name="cos_sb")
            sin_sb = const.tile([P, T], dt.bfloat16, tag="sin", name="sin_sb")
            mask_sb = const.tile([P, 2 * P], dt.bfloat16, tag="mask", name="mask_sb")
            esink_sb = const.tile([P, HPC], dt.float32, tag="esink", name="esink_sb")

            lnm = res.tile([P, T], dt.float32, tag="lnm", name="lnm")
            rsq_b = res.tile([P, T], dt.float32, tag="rsq", name="rsq_b")

            qra = res.tile([64, HPC, T], dt.bfloat16, tag="qra", name="qra")
            krope = res.tile([64, T], dt.bfloat16, tag="krope", name="krope")
            vtok = res.tile([P, MT, AW], dt.bfloat16, tag="vtok", name="vtok")
            nc.vector.memset(vtok[:, :, HD:HD + 1], 1.0)
            qkvT4 = res.tile([P, T], dt.bfloat16, tag="qkvT4", name="qkvT4")

            def dma(out, in_):
                nc.sync.dma_start(out=out, in_=in_)

            def dma2(out, in_):
                nc.scalar.dma_start(out=out, in_=in_)

            # ---- DMA issue, in dependency-need order ----
            # xsq half 0 (4 chunks of 6 k-tiles); ssq can start reducing early
            xsq_sb = [None, None]

            def load_xsq(half):
                c0 = half * 512
                xs = xsqp.tile([P, KSQ, 512], dt.float8e4, tag="xsq",
                               name=f"xsq{half}")
                for i in range(4):
                    (dma if i % 2 == 0 else dma2)(
                        xs[:, 6 * i:6 * (i + 1), :],
                        dram_tiles(xsq_d, 6 * i * P, 6, c0, 512))
                xsq_sb[half] = xs

            load_xsq(0)
            dma(wq_sb[4][:, :KPAD // 2], wq_d[4, :, :KPAD // 2])
            dma2(wq_sb[4][:, KPAD // 2:], wq_d[4, :, KPAD // 2:])
            dma2(cos_sb, cos_d)
            dma2(sin_sb, sin_d)
            dma2(mask_sb, mask_d)
            dma2(esink_sb, esink_d)
            # xt half 0: 6 chunks of 4(-ish) k-tiles alternating queues
            for half in range(2):
                c0 = half * 512
                if half == 1:
                    load_xsq(1)
                for i in range(6):
                    t0, t1 = 4 * i, min(4 * (i + 1), KT)
                    (dma if i % 2 == 0 else dma2)(
                        xt_sb[:, t0:t1, c0:c0 + 512],
                        dram_tiles(xt_d, t0 * P, t1 - t0, c0, 512))
                if half == 0:
                    for n in range(4):
                        (dma if n % 2 == 0 else dma2)(wq_sb[n], wq_d[n])
                    for kk in range(4):
                        (dma if kk % 2 == 0 else dma2)(wout_sb[kk],
                                                       wout_d[kk])

            # ---- ssq: fp8 DoubleRow ones-reduction, per token half ----
            psum_ssq = [None, None]

            def ssq_phase(half):
                ps = pY.tile([P, 512], dt.float32, tag="py", name=f"ssq{half}")
                xs = xsq_sb[half]
                for kk in range(KSQ // 2):
                    nc.tensor.matmul(
                        ps, ones8, xs[:, 2 * kk:2 * kk + 2, :],
                        start=(kk == 0), stop=(kk == KSQ // 2 - 1),
                        perf_mode=mybir.MatmulPerfMode.DoubleRow)
                psum_ssq[half] = ps

            def rsq_phase(half):
                c0 = half * 512
                sl = slice(c0, c0 + 512)
                nc.scalar.activation(lnm[:, sl], psum_ssq[half],
                                     mybir.ActivationFunctionType.Ln,
                                     bias=eps128, scale=1.0 / HIDDEN)
                nc.scalar.activation(rsq_b[:, sl], lnm[:, sl],
                                     mybir.ActivationFunctionType.Exp,
                                     bias=zbias, scale=-0.5)
                # bias row: xt[k=2880] = rms = 1/rsq so psum = Wx + b/rsq
                nc.scalar.activation(xt_sb[64:65, 22, sl], lnm[64:65, sl],
                                     mybir.ActivationFunctionType.Exp,
                                     bias=zbias[64:65, :], scale=0.5)

            # ---- qkv projection ----
            def qkv_tile(n, half, dst):
                """dst <- (W x + b/rsq)*rsq for columns [half*512, ...+512)."""
                c0 = half * 512
                pq = pA.tile([P, 512], dt.float32, tag="pb", name="pb")
                for ki in range(KT):
                    nc.tensor.matmul(
                        pq,
                        wq_sb[n][:, ki * P:ki * P + P],
                        xt_sb[:, ki, c0:c0 + 512],
                        start=(ki == 0), stop=(ki == KT - 1),
                    )
                nc.vector.tensor_mul(dst, pq, rsq_b[:, c0:c0 + 512])

            def kv_phase(half):
                c0 = half * 512
                sl = slice(c0, c0 + 512)
                qkv_tile(4, half, qkvT4[:, sl])
                for j in range(4):
                    b = half * 4 + j
                    pv = pG.tile([P, HD], dt.bfloat16, tag="pg", name="pv")
                    nc.tensor.transpose(pv, qkvT4[0:64, b * P:(b + 1) * P],
                                        identb[:64, :64])
                    nc.vector.tensor_copy(vtok[:, b, 0:HD], pv)
                # k rope (rows 64:128); sin table indexed at dst partition
                ktc = ropep.tile([P, 512], dt.bfloat16, tag="htc", name="ktc")
                nc.vector.tensor_mul(ktc[64:128, :], qkvT4[64:128, sl],
                                     cos_sb[64:128, sl])
                kts = ropep.tile([P, 512], dt.bfloat16, tag="hts", name="kts")
                for a in (64, 96):
                    b_ = a ^ 32
                    nc.vector.tensor_mul(kts[a:a + 32, :],
                                         qkvT4[b_:b_ + 32, sl],
                                         sin_sb[b_:b_ + 32, sl])
                nc.vector.tensor_add(krope[:, sl], ktc[64:128, :],
                                     kts[64:128, :])

            def q_phase(half, ns=range(4)):
                c0 = half * 512
                sl = slice(c0, c0 + 512)
                for n in ns:
                    qkvt = qkvp.tile([P, 512], dt.bfloat16, tag="qkvT",
                                     name=f"qkvt{n}")
                    qkv_tile(n, half, qkvt)
                    tc_ = ropep.tile([P, 512], dt.bfloat16, tag="htc", name="htc")
                    nc.vector.tensor_mul(tc_, qkvt, cos_sb[:, sl])
                    ts_ = ropep.tile([P, 512], dt.bfloat16, tag="hts", name="hts")
                    for a in range(0, P, 32):
                        b_ = a ^ 32
                        nc.vector.tensor_mul(ts_[a:a + 32, :],
                                             qkvt[b_:b_ + 32, :],
                                             sin_sb[b_:b_ + 32, sl])
                    for i in range(2):
                        b0 = 64 * i
                        nc.vector.tensor_add(qra[:, 2 * n + i, sl],
                                             tc_[b0:b0 + 64, :],
                                             ts_[b0:b0 + 64, :])

            # ---- attention front: scores .. att tile (out-proj lhsT) ----
            def attn_front(b):
                pt = ptp.tile([P, 2, HPC, P], dt.bfloat16, tag="pt", name="pt")
                kts = [(0, b - 1), (1, b)] if b > 0 else [(1, b)]
                rec8 = smallp.tile([P, HPC], dt.float32, tag="rec8", name="rec8")
                an = anp.tile([P, HPC, HD], dt.bfloat16, tag="an", name="an")
                att = attp.tile([P, 4, P], dt.bfloat16, tag="att", name="att")
                a2 = an.rearrange("p a b -> p (a b)")
                for g in range(2):
                    g0 = 4 * g
                    for s, kt in kts:
                        ps = pS.tile([P, 4, P], dt.float32, tag="sc", name="sc")
                        nc.tensor.matmul(
                            ps,
                            krope[:, kt * P:(kt + 1) * P],
                            qra[:, g0:g0 + 4, b * P:(b + 1) * P],
                            start=True, stop=True)
                        nc.scalar.activation(pt[:, s, g0:g0 + 4, :], ps,
                                             mybir.ActivationFunctionType.Exp,
                                             bias=zbias, scale=SM_SCALE)
                        m0 = 0 if s == 0 else P
                        nc.vector.tensor_mul(pt[:, s, g0:g0 + 4, :],
                                             pt[:, s, g0:g0 + 4, :],
                                             bcast_mid(mask_sb[:, m0:m0 + P], 4))
                    # AV with ones column -> denominators in column 64
                    pg = pG.tile([P, 4, AW], dt.float32, tag="pg", name="pg")
                    for j in range(4):
                        h = g0 + j
                        for idx, (s, kt) in enumerate(kts):
                            nc.tensor.matmul(pg[:, j, :], pt[:, s, h, :],
                                             vtok[:, kt, :],
                                             start=(idx == 0),
                                             stop=(idx == len(kts) - 1))
                    nc.vector.tensor_add(rec8[:, g0:g0 + 4],
                                         pg[:, :, HD:HD + 1],
                                         esink_sb[:, g0:g0 + 4])
                    nc.vector.reciprocal(rec8[:, g0:g0 + 4], rec8[:, g0:g0 + 4])
                    rec3 = bass.AP(tensor=rec8.tensor,
                                   offset=rec8[:, g0:g0 + 4].offset,
                                   ap=[rec8.ap[0], [1, 4], [0, HD]])
                    nc.vector.tensor_mul(an[:, g0:g0 + 4, :], pg[:, :, 0:HD],
                                         rec3)
                # transpose to out-proj lhsT layout [128 hd, 128 tok]
                for kk in range(4):
                    pat = pG.tile([P, P], dt.bfloat16, tag="pg", name="pat")
                    nc.tensor.transpose(pat, a2[:, kk * P:(kk + 1) * P],
                                        identb)
                    if kk % 2 == 0:
                        nc.vector.tensor_copy(att[:, kk, :], pat)
                    else:
                        nc.scalar.copy(att[:, kk, :], pat)
                return att

            # ---- out projection; y DMAs alternate queues ----
            def out_proj(b, att):
                for hy in range(3):
                    ysb = ysbp.tile([P, 2, YC], dt.bfloat16, tag="ysb",
                                    name="ysb")
                    for j in range(2):
                        ch = 2 * hy + j
                        o0 = ch * YC
                        pyt = pY.tile([P, 512], dt.float32, tag="py", name="py")
                        for kk in range(4):
                            nc.tensor.matmul(pyt[:, 0:YC], att[:, kk, :],
                                             wout_sb[kk][:, o0:o0 + YC],
                                             start=(kk == 0), stop=(kk == 3))
                        if ch % 2 == 0:
                            nc.vector.tensor_copy(ysb[:, j, :], pyt[:, 0:YC])
                        else:
                            nc.scalar.copy(ysb[:, j, :], pyt[:, 0:YC])
                        if b == MT - 1:
                            for q in range(2):
                                eng = nc.sync if q == 0 else nc.scalar
                                q0 = o0 + q * (YC // 2)
                                eng.dma_start(
                                    out=y_d[b * P:(b + 1) * P, q0:q0 + YC // 2],
                                    in_=ysb[:, j, q * (YC // 2):(q + 1) * (YC // 2)])
                    if b < MT - 1:
                        (dma if hy % 2 == 0 else dma2)(
                            y_d[b * P:(b + 1) * P, hy * 2 * YC:(hy + 1) * 2 * YC],
                            ysb)

            # ---- schedule ----
            ssq_phase(0)
            rsq_phase(0)
            kv_phase(0)
            q_phase(0)
            atts = {}
            atts[0] = attn_front(0)
            atts[1] = attn_front(1)
            out_proj(0, atts.pop(0))
            atts[2] = attn_front(2)
            out_proj(1, atts.pop(1))
            ssq_phase(1)
            rsq_phase(1)
            atts[3] = attn_front(3)
            out_proj(2, atts.pop(2))
            kv_phase(1)
            q_phase(1)
            atts[4] = attn_front(4)
            out_proj(3, atts.pop(3))
            for b in range(5, MT + 1):
                if b <= MT - 1:
                    atts[b] = attn_front(b)
                out_proj(b - 1, atts.pop(b - 1))

    nc.compile()
    return nc


# ----------------------------------------------------------------------------
# public entry
# ----------------------------------------------------------------------------

LAST_RESULTS = None


def kernel(x, norm_scale, qkv_w, qkv_b, out_w, out_b, sinks):
    global LAST_RESULTS
    x = np.asarray(x, dtype=np.float32)
    norm_scale = np.asarray(norm_scale, dtype=np.float32)
    qkv_w = np.asarray(qkv_w, dtype=np.float32)
    qkv_b = np.asarray(qkv_b, dtype=np.float32)
    out_w = np.asarray(out_w, dtype=np.float32)
    out_b = np.asarray(out_b, dtype=np.float32)
    sinks = np.asarray(sinks, dtype=np.float32)

    if "nc" not in _CACHE:
        _CACHE["nc"] = build_nc()
    nc = _CACHE["nc"]

    in_maps = [
        _prep_core_inputs(c, x, norm_scale, qkv_w, qkv_b, out_w, sinks)
        for c in range(NCORES)
    ]
    import os
    tmpdir = os.environ.get("BASS_TMPDIR") or None
    res = run_bass_kernel_spmd(nc, in_maps, core_ids=list(range(NCORES)),
                               tmpdir=tmpdir)
    LAST_RESULTS = res
    y = np.zeros((T, HIDDEN), dtype=np.float64)
    for c in range(NCORES):
        y += res.results[c]["y"].astype(np.float64)
    y += out_b.astype(np.float64)[None, :]
    return y.astype(np.float32)


# revision 25
# speedup vs baseline: 1.1575x; 1.0069x over previous
"""Trainium2 Bass kernel: sparse (sliding-window) attention block, v4.

Full module per reference:
  RMSNorm -> fused QKV (5120x2880) -> YaRN RoPE -> GQA sliding-window(128)
  causal attention with learned sink logit -> out projection (2880x4096).

Sharding: tensor-parallel over heads across 8 cores. Core c owns q-heads
[8c, 8c+8) and kv-head c. Each core emits a partial [1024, 2880] bf16
output; the host sums the partials (f64) and adds out_b.

v4 changes over v3 (all bf16 GEMMs kept -- fp8 matmuls were measured to
add ~8e-2 rel error through the logits, far over budget):
  - sum-of-squares via host-precomputed x^2 in fp8e4 + PE DoubleRow
    ones-reduction (2.6us PE instead of 9.8us PE + 10.8us DVE).
  - qkv bias folded into the matmul: xt row 2880 holds rms = 1/rsq
    (written on device), wq column k=2880 holds qkv_b, so
    psum = W x + b/rsq and one DVE mul by rsq finishes the tile.
  - rope tables in bf16 (DVE 2x mode) and indexed at the destination
    partition, so no separate negated-sin table is needed.
  - all rope sign-muls and mask muls on DVE (gpsimd ops measured
    1.1-1.7us each and serialized the attention pipeline in v3).
  - attention split into attn_front(b) (scores..att tile) and
    out_proj(b); out_proj(b-1) is emitted after attn_front(b) so its
    24 matmuls fill the exp/mask latency gaps on the PE.
"""

import math
import sys

import numpy as np

try:
    import concourse.bass as bass
except ImportError:  # pragma: no cover
    sys.path.insert(0, "/opt/trn_rl_repo")
    import concourse.bass as bass

import concourse.bacc as bacc
import concourse.tile as tile
from concourse import mybir
from concourse.masks import make_identity
from concourse.bass_utils import run_bass_kernel_spmd

import ml_dtypes

BF16 = ml_dtypes.bfloat16
F8 = ml_dtypes.float8_e4m3

T = 1024
HIDDEN = 2880
HD = 64
NH = 64
NKV = 8
SW = 128
NCORES = 8
HPC = NH // NCORES          # q heads per core = 8
QKV_DIM = HD * (NH + 2 * NKV)
SM_SCALE = 1.0 / math.sqrt(HD)

P = 128
KT = (HIDDEN + P - 1) // P   # 23 k-tiles over hidden (zero-padded to 2944)
KPAD = KT * P
KSQ = 24                     # xsq k-tiles (padded to an even count for DoubleRow)
NT = 5                       # qkv n-tiles of 128 (4 q-tiles + 1 kv-tile)
MT = T // P                  # 8 token tiles
AW = HD + 1                  # AV width: 64 v dims + denominator column
BIAS_K = HIDDEN              # contraction index carrying the bias (ki=22, kp=64)

dt = mybir.dt

_CACHE = {}


# ----------------------------------------------------------------------------
# host-side helpers
# ----------------------------------------------------------------------------

def _rope_cos_sin(num_tokens):
    base = 150000.0
    scaling = 32.0
    init_ctx = 4096.0
    ntk_alpha = 1.0
    ntk_beta = 32.0
    d_half = HD / 2
    freq = base ** (np.arange(0, HD, 2, dtype=np.float32) / HD)
    concentration = 0.1 * math.log(scaling) + 1.0
    low = d_half * math.log(init_ctx / (ntk_beta * 2 * math.pi)) / math.log(base)
    high = d_half * math.log(init_ctx / (ntk_alpha * 2 * math.pi)) / math.log(base)
    interpolation = 1.0 / (scaling * freq)
    extrapolation = 1.0 / freq
    ramp = (np.arange(int(d_half), dtype=np.float32) - low) / (high - low)
    m = 1.0 - np.clip(ramp, 0.0, 1.0)
    inv_freq = interpolation * (1.0 - m) + extrapolation * m
    t = np.arange(num_tokens, dtype=np.float32)
    freqs = t[:, None] * inv_freq[None, :]
    cos = (np.cos(freqs) * concentration).astype(np.float32)
    sin = (np.sin(freqs) * concentration).astype(np.float32)
    return cos, sin  # [T, 32]


def _host_tables():
    """Replicated rope tables, bf16, indexed at the DESTINATION partition:
    rope(u)[p] = u[p]*cos_t[p] + u[p^32]*sin_t[p]."""
    cos, sin = _rope_cos_sin(T)  # [1024, 32]
    sgn = np.repeat([-1.0, 1.0], 32)[:, None].astype(np.float32)
    sgn = np.tile(sgn, (2, 1))  # [128, 1]
    cos_t = np.tile(cos.T, (4, 1)).astype(BF16)          # [128, 1024]
    sin_t = (np.tile(sin.T, (4, 1)) * sgn).astype(BF16)  # [128, 1024]
    # walrus requires both SBUF inputs of a TensorTensor op to share the
    # base partition, so the sin table is indexed at the SOURCE partition:
    # ts[p] = u[p^32]*sin_t[p] = u[p^32]*sneg_t[p^32], sneg_t = -sin_t.
    return cos_t, (-sin_t.astype(np.float32)).astype(BF16)


def _host_masks01():
    j = np.arange(P)[:, None]   # kt row (partition)
    i = np.arange(P)[None, :]   # q col (free)
    mask_prev = (j > i).astype(np.float32)    # dist in [1,127]
    mask_self = (j <= i).astype(np.float32)   # dist in [0,127]
    return np.concatenate([mask_prev, mask_self], axis=1).astype(BF16)


def _prep_core_inputs(core, x, norm_scale, qkv_w, qkv_b, out_w, sinks):
    q_end = NH * HD
    k_end = q_end + NKV * HD

    qrows = np.arange(core * HPC * HD, (core + 1) * HPC * HD)
    krows = np.arange(q_end + core * HD, q_end + (core + 1) * HD)
    vrows = np.arange(k_end + core * HD, k_end + (core + 1) * HD)
    # kv n-tile: v in partitions 0:64, k in 64:128
    rows = np.concatenate([qrows, vrows, krows])  # [640]

    wshard = (qkv_w[rows, :] * norm_scale[None, :]).astype(np.float32)
    bshard = qkv_b[rows].astype(np.float32)  # [640]

    # lhsT tiles: wq[n, kp, ki*128 + nc] = wshard[n*128 + nc, ki*128 + kp];
    # the bias rides contraction index k=2880 (ki=22, kp=64) whose xt row is
    # filled with rms = 1/rsq on device.
    wq = np.zeros((NT, P, KPAD), dtype=BF16)
    for n in range(NT):
        blk = wshard[n * P:(n + 1) * P, :]  # [128 n, 2880 k]
        for ki in range(KT):
            k0 = ki * P
            ksz = min(P, HIDDEN - k0)
            wq[n, :ksz, ki * P:ki * P + P] = blk[:, k0:k0 + ksz].T.astype(BF16)
        wq[n, BIAS_K - 22 * P, 22 * P:23 * P] = bshard[n * P:(n + 1) * P].astype(BF16)

    cols = np.arange(core * HPC * HD, (core + 1) * HPC * HD)
    wo = out_w[:, cols].T.astype(np.float32)  # [512 hd, 2880 H]
    wout = wo.reshape(4, P, HIDDEN).astype(BF16)

    cos_t, sin_t = _host_tables()  # [128, 1024] bf16

    xt = np.zeros((KPAD, T), dtype=BF16)
    xt[:HIDDEN] = x.T.astype(BF16)

    xsq = np.zeros((KSQ * P, T), dtype=F8)
    xsq[:HIDDEN] = np.clip(x.T.astype(np.float64) ** 2, 0, 448).astype(F8)

    esink = np.exp(sinks[core * HPC:(core + 1) * HPC].astype(np.float64))
    esink = np.broadcast_to(esink.astype(np.float32), (P, HPC)).copy()

    return {
        "xt": xt,                               # [2944, 1024] bf16
        "xsq": xsq,                             # [3072, 1024] fp8e4
        "wq": wq,                               # [5, 128, 2944] bf16
        "wout": wout,                           # [4, 128, 2880] bf16
        "cos_t": cos_t, "sin_t": sin_t,         # [128, 1024] bf16
        "mask": _host_masks01(),                # [128, 256] bf16
        "esink": esink,                         # [128, 8] f32
    }


# ----------------------------------------------------------------------------
# device kernel (Tile)
# ----------------------------------------------------------------------------

def build_nc():
    nc = bacc.Bacc("TRN2", target_bir_lowering=False, debug=False)

    xt_d = nc.dram_tensor("xt", [KPAD, T], dt.bfloat16, kind="ExternalInput").ap()
    xsq_d = nc.dram_tensor("xsq", [KSQ * P, T], dt.float8e4,
                           kind="ExternalInput").ap()
    wq_d = nc.dram_tensor("wq", [NT, P, KPAD], dt.bfloat16, kind="ExternalInput").ap()
    wout_d = nc.dram_tensor("wout", [4, P, HIDDEN], dt.bfloat16,
                            kind="ExternalInput").ap()
    cos_d = nc.dram_tensor("cos_t", [P, T], dt.bfloat16, kind="ExternalInput").ap()
    sin_d = nc.dram_tensor("sin_t", [P, T], dt.bfloat16, kind="ExternalInput").ap()
    mask_d = nc.dram_tensor("mask", [P, 2 * P], dt.bfloat16, kind="ExternalInput").ap()
    esink_d = nc.dram_tensor("esink", [P, HPC], dt.float32, kind="ExternalInput").ap()
    y_d = nc.dram_tensor("y", [T, HIDDEN], dt.bfloat16, kind="ExternalOutput").ap()

    YC = 480

    def bcast_mid(ap2d, n):
        """[P, F] -> [P, n, F] with a 0-step middle dim (free broadcast)."""
        return bass.AP(tensor=ap2d.tensor, offset=ap2d.offset,
                       ap=[ap2d.ap[0], [0, n]] + list(ap2d.ap[1:]))

    def dram_tiles(d_ap, r0, nt, c0, csz):
        """[nt*128 rows from r0, cols c0:c0+csz] of a 2D dram tensor as a
        [128, nt, csz] AP (partition-major k-tiles)."""
        rstride = d_ap.ap[0][0]
        return bass.AP(tensor=d_ap.tensor, offset=d_ap.offset + r0 * rstride + c0,
                       ap=[[rstride, P], [P * rstride, nt], [1, csz]])

    with tile.TileContext(nc) as tc:
        with (
            tc.tile_pool(name="const", bufs=1) as const,
            tc.tile_pool(name="res", bufs=1) as res,
            tc.tile_pool(name="xsqp", bufs=2) as xsqp,
            tc.tile_pool(name="qkvp", bufs=2) as qkvp,
            tc.tile_pool(name="ropep", bufs=2) as ropep,
            tc.tile_pool(name="ptp", bufs=2) as ptp,
            tc.tile_pool(name="smallp", bufs=3) as smallp,
            tc.tile_pool(name="anp", bufs=2) as anp,
            tc.tile_pool(name="attp", bufs=2) as attp,
            tc.tile_pool(name="ysbp", bufs=3) as ysbp,
            tc.tile_pool(name="pA", bufs=2, space="PSUM") as pA,
            tc.tile_pool(name="pY", bufs=2, space="PSUM") as pY,
            tc.tile_pool(name="pS", bufs=2, space="PSUM") as pS,
            tc.tile_pool(name="pG", bufs=2, space="PSUM") as pG,
        ):
            # ---- constants ----
            zbias = const.tile([P, 1], dt.float32, tag="zbias", name="zbias")
            nc.vector.memset(zbias, 0.0)
            eps_t = const.tile([1, 1], dt.float32, tag="eps", name="eps_t")
            nc.vector.memset(eps_t, 1e-5)
            eps128 = const.tile([P, 1], dt.float32, tag="eps128", name="eps128")
            nc.vector.memset(eps128, 1e-5)
            # all-ones stationary [K, 2, 128]: every psum partition gets ssq,
            # so no partition_broadcast is needed for rsq.
            ones8 = const.tile([P, 2, P], dt.float8e4, tag="ones8", name="ones8")
            nc.vector.memset(ones8, 1.0)
            identb = const.tile([P, P], dt.bfloat16, tag="identb", name="identb")
            make_identity(nc, identb)
            dmy = const.tile([1, 2], dt.float32, tag="dmy", name="dmy")

            # ---- persistent SBUF ----
            wq_sb = [res.tile([P, KPAD], dt.bfloat16, tag=f"wq{n}", name=f"wq{n}")
                     for n in range(NT)]
            xt_sb = res.tile([P, KT, T], dt.bfloat16, tag="xt", name="xt")
            wout_sb = [res.tile([P, HIDDEN], dt.bfloat16, tag=f"wout{kk}",
                                name=f"wout{kk}") for kk in range(4)]
            cos_sb = const.tile([P, T], dt.bfloat16, tag="cos", name="cos_sb")
            sin_sb = const.tile([P, T], dt.bfloat16, tag="sin", name="sin_sb")
            mask_sb = const.tile([P, 2 * P], dt.bfloat16, tag="mask", name="mask_sb")
            esink_sb = const.tile([P, HPC], dt.float32, tag="esink", name="esink_sb")

            lnm = res.tile([P, T], dt.float32, tag="lnm", name="lnm")
            rsq_b = res.tile([P, T], dt.float32, tag="rsq", name="rsq_b")

            qra = res.tile([64, HPC, T], dt.bfloat16, tag="qra", name="qra")
            krope = res.tile([64, T], dt.bfloat16, tag="krope", name="krope")
            vtok = res.tile([P, MT, AW], dt.bfloat16, tag="vtok", name="vtok")
            nc.vector.memset(vtok[:, :, HD:HD + 1], 1.0)
            qkvT4 = res.tile([P, T], dt.bfloat16, tag="qkvT4", name="qkvT4")

            def dma(out, in_):
                nc.sync.dma_start(out=out, in_=in_)

            def dma2(out, in_):
                nc.scalar.dma_start(out=out, in_=in_)

            # ---- DMA issue, in dependency-need order ----
            # xsq (both halves) first: both rsq phases run up front, so no
            # activation-table swap or ssq stall lands mid-attention.
            xsq_sb = []
            for half in range(2):
                c0 = half * 512
                xs = xsqp.tile([P, KSQ, 512], dt.float8e4, tag="xsq",
                               name=f"xsq{half}")
                for i in range(2):
                    (dma if i % 2 == 0 else dma2)(
                        xs[:, 12 * i:12 * (i + 1), :],
                        dram_tiles(xsq_d, 12 * i * P, 12, c0, 512))
                xsq_sb.append(xs)
            dma(wq_sb[4][:, :KPAD // 2], wq_d[4, :, :KPAD // 2])
            dma2(wq_sb[4][:, KPAD // 2:], wq_d[4, :, KPAD // 2:])
            dma2(cos_sb, cos_d)
            dma2(sin_sb, sin_d)
            dma2(mask_sb, mask_d)
            dma2(esink_sb, esink_d)
            for i in range(6):  # xt half 0, in k-tile streaming order
                t0, t1 = 4 * i, min(4 * (i + 1), KT)
                (dma if i % 2 == 0 else dma2)(
                    xt_sb[:, t0:t1, 0:512],
                    dram_tiles(xt_d, t0 * P, t1 - t0, 0, 512))
            for n in range(4):
                (dma if n % 2 == 0 else dma2)(wq_sb[n], wq_d[n])
            for kk in range(4):
                (dma if kk % 2 == 0 else dma2)(wout_sb[kk], wout_d[kk])
            for i in range(6):  # xt half 1
                t0, t1 = 4 * i, min(4 * (i + 1), KT)
                (dma if i % 2 == 0 else dma2)(
                    xt_sb[:, t0:t1, 512:1024],
                    dram_tiles(xt_d, t0 * P, t1 - t0, 512, 512))



            # ---- ssq: fp8 DoubleRow ones-reduction, per token half ----
            psum_ssq = [None, None]

            def ssq_phase(half):
                ps = pY.tile([P, 512], dt.float32, tag="py", name=f"ssq{half}")
                xs = xsq_sb[half]
                for kk in range(KSQ // 2):
                    nc.tensor.matmul(
                        ps, ones8, xs[:, 2 * kk:2 * kk + 2, :],
                        start=(kk == 0), stop=(kk == KSQ // 2 - 1),
                        perf_mode=mybir.MatmulPerfMode.DoubleRow)
                psum_ssq[half] = ps

            def ln_phase(half):
                sl = slice(half * 512, half * 512 + 512)
                nc.scalar.activation(lnm[:, sl], psum_ssq[half],
                                     mybir.ActivationFunctionType.Ln,
                                     bias=eps128, scale=1.0 / HIDDEN)

            def rsq_phase(half):
                sl = slice(half * 512, half * 512 + 512)
                nc.scalar.activation(rsq_b[:, sl], lnm[:, sl],
                                     mybir.ActivationFunctionType.Exp,
                                     bias=zbias, scale=-0.5)
                # bias row: xt[k=2880] = rms = 1/rsq so psum = Wx + b/rsq
                # (bf16 out is fine: it only scales the small qkv bias)
                with nc.allow_low_precision(reason="rms bias row, bf16 ok"):
                    nc.vector.reciprocal(xt_sb[64:65, 22, sl],
                                         rsq_b[64:65, sl])

            # ---- qkv projection ----
            def qkv_tile(n, half, dst):
                """dst <- (W x + b/rsq)*rsq for columns [half*512, ...+512)."""
                c0 = half * 512
                pq = pA.tile([P, 512], dt.float32, tag="pb", name="pb")
                for ki in range(KT):
                    nc.tensor.matmul(
                        pq,
                        wq_sb[n][:, ki * P:ki * P + P],
                        xt_sb[:, ki, c0:c0 + 512],
                        start=(ki == 0), stop=(ki == KT - 1),
                    )
                nc.vector.tensor_mul(dst, pq, rsq_b[:, c0:c0 + 512])

            def kv_phase(half):
                c0 = half * 512
                sl = slice(c0, c0 + 512)
                qkv_tile(4, half, qkvT4[:, sl])
                for j in range(4):
                    b = half * 4 + j
                    pv = pG.tile([P, HD], dt.bfloat16, tag="pg", name="pv")
                    nc.tensor.transpose(pv, qkvT4[0:64, b * P:(b + 1) * P],
                                        identb[:64, :64])
                    nc.vector.tensor_copy(vtok[:, b, 0:HD], pv)
                # k rope (rows 64:128); sin table indexed at dst partition
                ktc = ropep.tile([P, 512], dt.bfloat16, tag="htc", name="ktc")
                nc.vector.tensor_mul(ktc[64:128, :], qkvT4[64:128, sl],
                                     cos_sb[64:128, sl])
                kts = ropep.tile([P, 512], dt.bfloat16, tag="hts", name="kts")
                for a in (64, 96):
                    b_ = a ^ 32
                    nc.vector.tensor_mul(kts[a:a + 32, :],
                                         qkvT4[b_:b_ + 32, sl],
                                         sin_sb[b_:b_ + 32, sl])
                nc.vector.tensor_add(krope[:, sl], ktc[64:128, :],
                                     kts[64:128, :])

            def q_phase(half, ns=range(4)):
                c0 = half * 512
                sl = slice(c0, c0 + 512)
                for n in ns:
                    qkvt = qkvp.tile([P, 512], dt.bfloat16, tag="qkvT",
                                     name=f"qkvt{n}")
                    qkv_tile(n, half, qkvt)
                    tc_ = ropep.tile([P, 512], dt.bfloat16, tag="htc", name="htc")
                    nc.vector.tensor_mul(tc_, qkvt, cos_sb[:, sl])
                    ts_ = ropep.tile([P, 512], dt.bfloat16, tag="hts", name="hts")
                    for a in range(0, P, 32):
                        b_ = a ^ 32
                        nc.vector.tensor_mul(ts_[a:a + 32, :],
                                             qkvt[b_:b_ + 32, :],
                                             sin_sb[b_:b_ + 32, sl])
                    for i in range(2):
                        b0 = 64 * i
                        nc.vector.tensor_add(qra[:, 2 * n + i, sl],
                                             tc_[b0:b0 + 64, :],
                                             ts_[b0:b0 + 64, :])

            # ---- attention front: scores .. att tile (out-proj lhsT) ----
            def attn_front(b):
                pt = ptp.tile([P, 2, HPC, P], dt.bfloat16, tag="pt", name="pt")
                kts = [(0, b - 1), (1, b)] if b > 0 else [(1, b)]
                rec8 = smallp.tile([P, HPC], dt.float32, tag="rec8", name="rec8")
                an = anp.tile([P, HPC, HD], dt.bfloat16, tag="an", name="an")
                att = attp.tile([P, 4, P], dt.bfloat16, tag="att", name="att")
                a2 = an.rearrange("p a b -> p (a b)")
                for g in range(2):
                    g0 = 4 * g
                    for s, kt in kts:
                        ps = pS.tile([P, 4, P], dt.float32, tag="sc", name="sc")
                        nc.tensor.matmul(
                            ps,
                            krope[:, kt * P:(kt + 1) * P],
                            qra[:, g0:g0 + 4, b * P:(b + 1) * P],
                            start=True, stop=True)
                        nc.scalar.activation(pt[:, s, g0:g0 + 4, :], ps,
                                             mybir.ActivationFunctionType.Exp,
                                             bias=zbias, scale=SM_SCALE)
                        m0 = 0 if s == 0 else P
                        nc.vector.tensor_mul(pt[:, s, g0:g0 + 4, :],
                                             pt[:, s, g0:g0 + 4, :],
                                             bcast_mid(mask_sb[:, m0:m0 + P], 4))
                    # AV with ones column -> denominators in column 64
                    pg = pG.tile([P, 4, AW], dt.float32, tag="pg", name="pg")
                    for j in range(4):
                        h = g0 + j
                        for idx, (s, kt) in enumerate(kts):
                            nc.tensor.matmul(pg[:, j, :], pt[:, s, h, :],
                                             vtok[:, kt, :],
                                             start=(idx == 0),
                                             stop=(idx == len(kts) - 1))
                    nc.vector.tensor_add(rec8[:, g0:g0 + 4],
                                         pg[:, :, HD:HD + 1],
                                         esink_sb[:, g0:g0 + 4])
                    nc.vector.reciprocal(rec8[:, g0:g0 + 4], rec8[:, g0:g0 + 4])
                    rec3 = bass.AP(tensor=rec8.tensor,
                                   offset=rec8[:, g0:g0 + 4].offset,
                                   ap=[rec8.ap[0], [1, 4], [0, HD]])
                    nc.vector.tensor_mul(an[:, g0:g0 + 4, :], pg[:, :, 0:HD],
                                         rec3)
                # transpose to out-proj lhsT layout [128 hd, 128 tok]
                for kk in range(4):
                    pat = pG.tile([P, P], dt.bfloat16, tag="pg", name="pat")
                    nc.tensor.transpose(pat, a2[:, kk * P:(kk + 1) * P],
                                        identb)
                    if kk % 2 == 0:
                        nc.vector.tensor_copy(att[:, kk, :], pat)
                    else:
                        nc.scalar.copy(att[:, kk, :], pat)
                return att

            # ---- out projection; one y DMA per tile (finer on the last) ----
            def out_proj(b, att):
                ysb = ysbp.tile([P, 6, YC], dt.bfloat16, tag="ysb", name="ysb")
                for ch in range(6):
                    o0 = ch * YC
                    pyt = pY.tile([P, 512], dt.float32, tag="py", name="py")
                    for kk in range(4):
                        nc.tensor.matmul(pyt[:, 0:YC], att[:, kk, :],
                                         wout_sb[kk][:, o0:o0 + YC],
                                         start=(kk == 0), stop=(kk == 3))
                    if ch % 2 == 0:
                        nc.vector.tensor_copy(ysb[:, ch, :], pyt[:, 0:YC])
                    else:
                        nc.scalar.copy(ysb[:, ch, :], pyt[:, 0:YC])
                    if b == MT - 1:
                        for q in range(2):
                            eng = nc.sync if (ch + q) % 2 == 0 else nc.scalar
                            q0 = o0 + q * (YC // 2)
                            eng.dma_start(
                                out=y_d[b * P:(b + 1) * P, q0:q0 + YC // 2],
                                in_=ysb[:, ch, q * (YC // 2):(q + 1) * (YC // 2)])
                if b < MT - 1:
                    (dma if b % 2 == 0 else dma2)(y_d[b * P:(b + 1) * P, :], ysb)

            # ---- schedule ----
            ssq_phase(0)
            ssq_phase(1)
            # both Lns, then both Exps: each activation-table set loads
            # exactly once, early; attention then stays on the exp set.
            ln_phase(0)
            ln_phase(1)
            rsq_phase(0)
            rsq_phase(1)
            kv_phase(0)
            q_phase(0)
            atts = {}
            atts[0] = attn_front(0)
            atts[1] = attn_front(1)
            out_proj(0, atts.pop(0))
            atts[2] = attn_front(2)
            out_proj(1, atts.pop(1))
            atts[3] = attn_front(3)
            out_proj(2, atts.pop(2))
            kv_phase(1)
            q_phase(1)
            atts[4] = attn_front(4)
            out_proj(3, atts.pop(3))
            for b in range(5, MT + 1):
                if b <= MT - 1:
                    atts[b] = attn_front(b)
                out_proj(b - 1, atts.pop(b - 1))

    nc.compile()
    return nc


# ----------------------------------------------------------------------------
# public entry
# ----------------------------------------------------------------------------

LAST_RESULTS = None


def kernel(x, norm_scale, qkv_w, qkv_b, out_w, out_b, sinks):
    global LAST_RESULTS
    x = np.asarray(x, dtype=np.float32)
    norm_scale = np.asarray(norm_scale, dtype=np.float32)
    qkv_w = np.asarray(qkv_w, dtype=np.float32)
    qkv_b = np.asarray(qkv_b, dtype=np.float32)
    out_w = np.asarray(out_w, dtype=np.float32)
    out_b = np.asarray(out_b, dtype=np.float32)
    sinks = np.asarray(sinks, dtype=np.float32)

    if "nc" not in _CACHE:
        _CACHE["nc"] = build_nc()
    nc = _CACHE["nc"]

    in_maps = [
        _prep_core_inputs(c, x, norm_scale, qkv_w, qkv_b, out_w, sinks)
        for c in range(NCORES)
    ]
    import os
    tmpdir = os.environ.get("BASS_TMPDIR") or None
    res = run_bass_kernel_spmd(nc, in_maps, core_ids=list(range(NCORES)),
                               tmpdir=tmpdir)
    LAST_RESULTS = res
    y = np.zeros((T, HIDDEN), dtype=np.float64)
    for c in range(NCORES):
        y += res.results[c]["y"].astype(np.float64)
    y += out_b.astype(np.float64)[None, :]
    return y.astype(np.float32)


# revision 28
# speedup vs baseline: 1.2441x; 1.0748x over previous
"""Trainium2 Bass kernel: sparse (sliding-window) attention block, v4.

Full module per reference:
  RMSNorm -> fused QKV (5120x2880) -> YaRN RoPE -> GQA sliding-window(128)
  causal attention with learned sink logit -> out projection (2880x4096).

Sharding: tensor-parallel over heads across 8 cores. Core c owns q-heads
[8c, 8c+8) and kv-head c. Each core emits a partial [1024, 2880] bf16
output; the host sums the partials (f64) and adds out_b.

v4 changes over v3 (all bf16 GEMMs kept -- fp8 matmuls were measured to
add ~8e-2 rel error through the logits, far over budget):
  - sum-of-squares via host-precomputed x^2 in fp8e4 + PE DoubleRow
    ones-reduction (2.6us PE instead of 9.8us PE + 10.8us DVE).
  - qkv bias folded into the matmul: xt row 2880 holds rms = 1/rsq
    (written on device), wq column k=2880 holds qkv_b, so
    psum = W x + b/rsq and one DVE mul by rsq finishes the tile.
  - rope tables in bf16 (DVE 2x mode) and indexed at the destination
    partition, so no separate negated-sin table is needed.
  - all rope sign-muls and mask muls on DVE (gpsimd ops measured
    1.1-1.7us each and serialized the attention pipeline in v3).
  - attention split into attn_front(b) (scores..att tile) and
    out_proj(b); out_proj(b-1) is emitted after attn_front(b) so its
    24 matmuls fill the exp/mask latency gaps on the PE.
"""

import math
import sys

import numpy as np

try:
    import concourse.bass as bass
except ImportError:  # pragma: no cover
    sys.path.insert(0, "/opt/trn_rl_repo")
    import concourse.bass as bass

import concourse.bacc as bacc
import concourse.tile as tile
from concourse import mybir
from concourse.masks import make_identity
from concourse.bass_utils import run_bass_kernel_spmd

import ml_dtypes

BF16 = ml_dtypes.bfloat16
F8 = ml_dtypes.float8_e4m3

T = 1024
HIDDEN = 2880
HD = 64
NH = 64
NKV = 8
SW = 128
NCORES = 8
HPC = NH // NCORES          # q heads per core = 8
QKV_DIM = HD * (NH + 2 * NKV)
SM_SCALE = 1.0 / math.sqrt(HD)

P = 128
KT = (HIDDEN + P - 1) // P   # 23 k-tiles over hidden (zero-padded to 2944)
KPAD = KT * P
KSQ = 24                     # xsq k-tiles (padded to an even count for DoubleRow)
NT = 5                       # qkv n-tiles of 128 (4 q-tiles + 1 kv-tile)
MT = T // P                  # 8 token tiles
AW = HD + 1                  # AV width: 64 v dims + denominator column
BIAS_K = HIDDEN              # contraction index carrying the bias (ki=22, kp=64)

dt = mybir.dt

_CACHE = {}


# ----------------------------------------------------------------------------
# host-side helpers
# ----------------------------------------------------------------------------

def _rope_cos_sin(num_tokens):
    base = 150000.0
    scaling = 32.0
    init_ctx = 4096.0
    ntk_alpha = 1.0
    ntk_beta = 32.0
    d_half = HD / 2
    freq = base ** (np.arange(0, HD, 2, dtype=np.float32) / HD)
    concentration = 0.1 * math.log(scaling) + 1.0
    low = d_half * math.log(init_ctx / (ntk_beta * 2 * math.pi)) / math.log(base)
    high = d_half * math.log(init_ctx / (ntk_alpha * 2 * math.pi)) / math.log(base)
    interpolation = 1.0 / (scaling * freq)
    extrapolation = 1.0 / freq
    ramp = (np.arange(int(d_half), dtype=np.float32) - low) / (high - low)
    m = 1.0 - np.clip(ramp, 0.0, 1.0)
    inv_freq = interpolation * (1.0 - m) + extrapolation * m
    t = np.arange(num_tokens, dtype=np.float32)
    freqs = t[:, None] * inv_freq[None, :]
    cos = (np.cos(freqs) * concentration).astype(np.float32)
    sin = (np.sin(freqs) * concentration).astype(np.float32)
    return cos, sin  # [T, 32]


def _host_tables():
    """Replicated rope tables, bf16, indexed at the DESTINATION partition:
    rope(u)[p] = u[p]*cos_t[p] + u[p^32]*sin_t[p]."""
    cos, sin = _rope_cos_sin(T)  # [1024, 32]
    sgn = np.repeat([-1.0, 1.0], 32)[:, None].astype(np.float32)
    sgn = np.tile(sgn, (2, 1))  # [128, 1]
    cos_t = np.tile(cos.T, (4, 1)).astype(BF16)          # [128, 1024]
    sin_t = (np.tile(sin.T, (4, 1)) * sgn).astype(BF16)  # [128, 1024]
    # walrus requires both SBUF inputs of a TensorTensor op to share the
    # base partition, so the sin table is indexed at the SOURCE partition:
    # ts[p] = u[p^32]*sin_t[p] = u[p^32]*sneg_t[p^32], sneg_t = -sin_t.
    return cos_t, (-sin_t.astype(np.float32)).astype(BF16)


def _host_masks01():
    j = np.arange(P)[:, None]   # kt row (partition)
    i = np.arange(P)[None, :]   # q col (free)
    mask_prev = (j > i).astype(np.float32)    # dist in [1,127]
    mask_self = (j <= i).astype(np.float32)   # dist in [0,127]
    return np.concatenate([mask_prev, mask_self], axis=1).astype(BF16)


def _prep_core_inputs(core, x, norm_scale, qkv_w, qkv_b, out_w, sinks):
    q_end = NH * HD
    k_end = q_end + NKV * HD

    qrows = np.arange(core * HPC * HD, (core + 1) * HPC * HD)
    krows = np.arange(q_end + core * HD, q_end + (core + 1) * HD)
    vrows = np.arange(k_end + core * HD, k_end + (core + 1) * HD)
    # kv n-tile: v in partitions 0:64, k in 64:128
    rows = np.concatenate([qrows, vrows, krows])  # [640]

    wshard = (qkv_w[rows, :] * norm_scale[None, :]).astype(np.float32)
    bshard = qkv_b[rows].astype(np.float32)  # [640]

    # lhsT tiles: wq[n, kp, ki*128 + nc] = wshard[n*128 + nc, ki*128 + kp];
    # the bias rides contraction index k=2880 (ki=22, kp=64) whose xt row is
    # filled with rms = 1/rsq on device.
    wq = np.zeros((NT, P, KPAD), dtype=BF16)
    for n in range(NT):
        blk = wshard[n * P:(n + 1) * P, :]  # [128 n, 2880 k]
        for ki in range(KT):
            k0 = ki * P
            ksz = min(P, HIDDEN - k0)
            wq[n, :ksz, ki * P:ki * P + P] = blk[:, k0:k0 + ksz].T.astype(BF16)
        wq[n, BIAS_K - 22 * P, 22 * P:23 * P] = bshard[n * P:(n + 1) * P].astype(BF16)

    cols = np.arange(core * HPC * HD, (core + 1) * HPC * HD)
    wo = out_w[:, cols].T.astype(np.float32)  # [512 hd, 2880 H]
    wout = wo.reshape(4, P, HIDDEN).astype(BF16)

    cos_t, sin_t = _host_tables()  # [128, 1024] bf16

    xt = np.zeros((KPAD, T), dtype=BF16)
    xt[:HIDDEN] = x.T.astype(BF16)

    xsq = np.zeros((KSQ * P, T), dtype=F8)
    xsq[:HIDDEN] = np.clip(x.T.astype(np.float64) ** 2, 0, 448).astype(F8)

    esink = np.exp(sinks[core * HPC:(core + 1) * HPC].astype(np.float64))
    esink = np.broadcast_to(esink.astype(np.float32), (P, HPC)).copy()

    return {
        "xt": xt,                               # [2944, 1024] bf16
        "xsq": xsq,                             # [3072, 1024] fp8e4
        "wq": wq,                               # [5, 128, 2944] bf16
        "wout": wout,                           # [4, 128, 2880] bf16
        "cos_t": cos_t, "sin_t": sin_t,         # [128, 1024] bf16
        "mask": _host_masks01(),                # [128, 256] bf16
        "esink": esink,                         # [128, 8] f32
    }


# ----------------------------------------------------------------------------
# device kernel (Tile)
# ----------------------------------------------------------------------------

def build_nc():
    nc = bacc.Bacc("TRN2", target_bir_lowering=False, debug=False)

    xt_d = nc.dram_tensor("xt", [KPAD, T], dt.bfloat16, kind="ExternalInput").ap()
    xsq_d = nc.dram_tensor("xsq", [KSQ * P, T], dt.float8e4,
                           kind="ExternalInput").ap()
    wq_d = nc.dram_tensor("wq", [NT, P, KPAD], dt.bfloat16, kind="ExternalInput").ap()
    wout_d = nc.dram_tensor("wout", [4, P, HIDDEN], dt.bfloat16,
                            kind="ExternalInput").ap()
    cos_d = nc.dram_tensor("cos_t", [P, T], dt.bfloat16, kind="ExternalInput").ap()
    sin_d = nc.dram_tensor("sin_t", [P, T], dt.bfloat16, kind="ExternalInput").ap()
    mask_d = nc.dram_tensor("mask", [P, 2 * P], dt.bfloat16, kind="ExternalInput").ap()
    esink_d = nc.dram_tensor("esink", [P, HPC], dt.float32, kind="ExternalInput").ap()
    y_d = nc.dram_tensor("y", [T, HIDDEN], dt.bfloat16, kind="ExternalOutput").ap()

    YC = 480

    def bcast_mid(ap2d, n):
        """[P, F] -> [P, n, F] with a 0-step middle dim (free broadcast)."""
        return bass.AP(tensor=ap2d.tensor, offset=ap2d.offset,
                       ap=[ap2d.ap[0], [0, n]] + list(ap2d.ap[1:]))

    def dram_tiles(d_ap, r0, nt, c0, csz):
        """[nt*128 rows from r0, cols c0:c0+csz] of a 2D dram tensor as a
        [128, nt, csz] AP (partition-major k-tiles)."""
        rstride = d_ap.ap[0][0]
        return bass.AP(tensor=d_ap.tensor, offset=d_ap.offset + r0 * rstride + c0,
                       ap=[[rstride, P], [P * rstride, nt], [1, csz]])

    with tile.TileContext(nc) as tc:
        with (
            tc.tile_pool(name="const", bufs=1) as const,
            tc.tile_pool(name="res", bufs=1) as res,
            tc.tile_pool(name="xsqp", bufs=2) as xsqp,
            tc.tile_pool(name="qkvp", bufs=2) as qkvp,
            tc.tile_pool(name="ropep", bufs=2) as ropep,
            tc.tile_pool(name="ptp", bufs=2) as ptp,
            tc.tile_pool(name="smallp", bufs=3) as smallp,
            tc.tile_pool(name="anp", bufs=2) as anp,
            tc.tile_pool(name="attp", bufs=2) as attp,
            tc.tile_pool(name="ysbp", bufs=3) as ysbp,
            tc.tile_pool(name="pA", bufs=2, space="PSUM") as pA,
            tc.tile_pool(name="pY", bufs=2, space="PSUM") as pY,
            tc.tile_pool(name="pS", bufs=2, space="PSUM") as pS,
            tc.tile_pool(name="pG", bufs=2, space="PSUM") as pG,
        ):
            # ---- constants ----
            zbias = const.tile([P, 1], dt.float32, tag="zbias", name="zbias")
            nc.vector.memset(zbias, 0.0)
            eps_t = const.tile([1, 1], dt.float32, tag="eps", name="eps_t")
            nc.vector.memset(eps_t, 1e-5)
            eps128 = const.tile([P, 1], dt.float32, tag="eps128", name="eps128")
            nc.vector.memset(eps128, 1e-5)
            # all-ones stationary [K, 2, 128]: every psum partition gets ssq,
            # so no partition_broadcast is needed for rsq.
            ones8 = const.tile([P, 2, P], dt.float8e4, tag="ones8", name="ones8")
            nc.vector.memset(ones8, 1.0)
            identb = const.tile([P, P], dt.bfloat16, tag="identb", name="identb")
            make_identity(nc, identb)
            dmy = const.tile([1, 2], dt.float32, tag="dmy", name="dmy")

            # ---- persistent SBUF ----
            wqall = res.tile([P, 4, KPAD], dt.bfloat16, tag="wqall", name="wqall")
            wq4t = res.tile([P, KPAD], dt.bfloat16, tag="wq4", name="wq4")
            wq_sb = [wqall[:, n, :] for n in range(4)] + [wq4t]
            xt_sb = res.tile([P, KT, T], dt.bfloat16, tag="xt", name="xt")
            woutall = res.tile([P, 4, HIDDEN], dt.bfloat16, tag="woutall",
                               name="woutall")
            wout_sb = [woutall[:, kk, :] for kk in range(4)]
            cos_sb = const.tile([P, T], dt.bfloat16, tag="cos", name="cos_sb")
            sin_sb = const.tile([P, T], dt.bfloat16, tag="sin", name="sin_sb")
            mask_sb = const.tile([P, 2 * P], dt.bfloat16, tag="mask", name="mask_sb")
            esink_sb = const.tile([P, HPC], dt.float32, tag="esink", name="esink_sb")

            lnm = res.tile([P, T], dt.float32, tag="lnm", name="lnm")
            rsq_b = res.tile([P, T], dt.float32, tag="rsq", name="rsq_b")

            qra = res.tile([64, HPC, T], dt.bfloat16, tag="qra", name="qra")
            krope = res.tile([64, T], dt.bfloat16, tag="krope", name="krope")
            vtok = res.tile([P, MT, AW], dt.bfloat16, tag="vtok", name="vtok")
            nc.vector.memset(vtok[:, :, HD:HD + 1], 1.0)
            qkvT4 = res.tile([P, T], dt.bfloat16, tag="qkvT4", name="qkvT4")

            def dma(out, in_):
                nc.sync.dma_start(out=out, in_=in_)

            def dma2(out, in_):
                nc.scalar.dma_start(out=out, in_=in_)

            # ---- DMA issue ----
            # All bulk loads ride the sync queue; the scalar queue stays free
            # for the rsq activations and attention work (DMA-ring semaphore
            # waits on a clogged queue were measured delaying the queued
            # activations by 35us).
            xsq_sb = []
            for half in range(2):
                xs = xsqp.tile([P, KSQ, 512], dt.float8e4, tag="xsq",
                               name=f"xsq{half}")
                dma(xs, dram_tiles(xsq_d, 0, KSQ, half * 512, 512))
                xsq_sb.append(xs)
            dma2(cos_sb, cos_d)
            dma2(sin_sb, sin_d)
            dma2(mask_sb, mask_d)
            dma2(esink_sb, esink_d)
            dma(wq4t, wq_d[4])

            def wq_pair(n0):
                dma(wqall[:, n0:n0 + 2, :],
                    bass.AP(tensor=wq_d.tensor,
                            offset=wq_d.offset + n0 * P * KPAD,
                            ap=[[KPAD, P], [P * KPAD, 2], [1, KPAD]]))

            def xt_chunk(i, half):
                t0, t1 = 8 * i, min(8 * (i + 1), KT)
                dma(xt_sb[:, t0:t1, half * 512:half * 512 + 512],
                    dram_tiles(xt_d, t0 * P, t1 - t0, half * 512, 512))

            xt_chunk(0, 0)
            xt_chunk(1, 0)
            wq_pair(0)
            xt_chunk(2, 0)
            wq_pair(2)
            dma(woutall, bass.AP(tensor=wout_d.tensor, offset=wout_d.offset,
                                 ap=[[HIDDEN, P], [P * HIDDEN, 4], [1, HIDDEN]]))
            for i in range(3):  # xt half 1
                xt_chunk(i, 1)



            # ---- ssq: fp8 DoubleRow ones-reduction, per token half ----
            psum_ssq = [None, None]

            def ssq_phase(half):
                ps = pY.tile([P, 512], dt.float32, tag="py", name=f"ssq{half}")
                xs = xsq_sb[half]
                for kk in range(KSQ // 2):
                    nc.tensor.matmul(
                        ps, ones8, xs[:, 2 * kk:2 * kk + 2, :],
                        start=(kk == 0), stop=(kk == KSQ // 2 - 1),
                        perf_mode=mybir.MatmulPerfMode.DoubleRow)
                psum_ssq[half] = ps

            def ln_phase(half):
                sl = slice(half * 512, half * 512 + 512)
                nc.scalar.activation(lnm[:, sl], psum_ssq[half],
                                     mybir.ActivationFunctionType.Ln,
                                     bias=eps128, scale=1.0 / HIDDEN)

            def rsq_phase(half):
                sl = slice(half * 512, half * 512 + 512)
                nc.scalar.activation(rsq_b[:, sl], lnm[:, sl],
                                     mybir.ActivationFunctionType.Exp,
                                     bias=zbias, scale=-0.5)
                # bias row: xt[k=2880] = rms = 1/rsq so psum = Wx + b/rsq
                # (bf16 out is fine: it only scales the small qkv bias)
                with nc.allow_low_precision(reason="rms bias row, bf16 ok"):
                    nc.vector.reciprocal(xt_sb[64:65, 22, sl],
                                         rsq_b[64:65, sl])

            # ---- qkv projection ----
            def qkv_tile(n, half, dst):
                """dst <- (W x + b/rsq)*rsq for columns [half*512, ...+512)."""
                c0 = half * 512
                pq = pA.tile([P, 512], dt.float32, tag="pb", name="pb")
                for ki in range(KT):
                    nc.tensor.matmul(
                        pq,
                        wq_sb[n][:, ki * P:ki * P + P],
                        xt_sb[:, ki, c0:c0 + 512],
                        start=(ki == 0), stop=(ki == KT - 1),
                    )
                nc.vector.tensor_mul(dst, pq, rsq_b[:, c0:c0 + 512])

            def kv_phase(half):
                c0 = half * 512
                sl = slice(c0, c0 + 512)
                qkv_tile(4, half, qkvT4[:, sl])
                for j in range(4):
                    b = half * 4 + j
                    pv = pG.tile([P, HD], dt.bfloat16, tag="pg", name="pv")
                    nc.tensor.transpose(pv, qkvT4[0:64, b * P:(b + 1) * P],
                                        identb[:64, :64])
                    nc.vector.tensor_copy(vtok[:, b, 0:HD], pv)
                # k rope (rows 64:128); sin table indexed at dst partition
                ktc = ropep.tile([P, 512], dt.bfloat16, tag="htc", name="ktc")
                nc.vector.tensor_mul(ktc[64:128, :], qkvT4[64:128, sl],
                                     cos_sb[64:128, sl])
                kts = ropep.tile([P, 512], dt.bfloat16, tag="hts", name="kts")
                for a in (64, 96):
                    b_ = a ^ 32
                    nc.vector.tensor_mul(kts[a:a + 32, :],
                                         qkvT4[b_:b_ + 32, sl],
                                         sin_sb[b_:b_ + 32, sl])
                nc.vector.tensor_add(krope[:, sl], ktc[64:128, :],
                                     kts[64:128, :])

            def q_phase(half, ns=range(4)):
                c0 = half * 512
                sl = slice(c0, c0 + 512)
                for n in ns:
                    qkvt = qkvp.tile([P, 512], dt.bfloat16, tag="qkvT",
                                     name=f"qkvt{n}")
                    qkv_tile(n, half, qkvt)
                    tc_ = ropep.tile([P, 512], dt.bfloat16, tag="htc", name="htc")
                    nc.vector.tensor_mul(tc_, qkvt, cos_sb[:, sl])
                    ts_ = ropep.tile([P, 512], dt.bfloat16, tag="hts", name="hts")
                    for a in range(0, P, 32):
                        b_ = a ^ 32
                        nc.vector.tensor_mul(ts_[a:a + 32, :],
                                             qkvt[b_:b_ + 32, :],
                                             sin_sb[b_:b_ + 32, sl])
                    for i in range(2):
                        b0 = 64 * i
                        nc.vector.tensor_add(qra[:, 2 * n + i, sl],
                                             tc_[b0:b0 + 64, :],
                                             ts_[b0:b0 + 64, :])

            # ---- attention front: scores .. att tile (out-proj lhsT) ----
            def attn_front(b):
                pt = ptp.tile([P, 2, HPC, P], dt.bfloat16, tag="pt", name="pt")
                kts = [(0, b - 1), (1, b)] if b > 0 else [(1, b)]
                rec8 = smallp.tile([P, HPC], dt.float32, tag="rec8", name="rec8")
                an = anp.tile([P, HPC, HD], dt.bfloat16, tag="an", name="an")
                att = attp.tile([P, 4, P], dt.bfloat16, tag="att", name="att")
                a2 = an.rearrange("p a b -> p (a b)")
                for g in range(2):
                    g0 = 4 * g
                    for s, kt in kts:
                        ps = pS.tile([P, 4, P], dt.float32, tag="sc", name="sc")
                        nc.tensor.matmul(
                            ps,
                            krope[:, kt * P:(kt + 1) * P],
                            qra[:, g0:g0 + 4, b * P:(b + 1) * P],
                            start=True, stop=True)
                        nc.scalar.activation(pt[:, s, g0:g0 + 4, :], ps,
                                             mybir.ActivationFunctionType.Exp,
                                             bias=zbias, scale=SM_SCALE)
                        m0 = 0 if s == 0 else P
                        nc.vector.tensor_mul(pt[:, s, g0:g0 + 4, :],
                                             pt[:, s, g0:g0 + 4, :],
                                             bcast_mid(mask_sb[:, m0:m0 + P], 4))
                    # AV with ones column -> denominators in column 64
                    pg = pG.tile([P, 4, AW], dt.float32, tag="pg", name="pg")
                    for j in range(4):
                        h = g0 + j
                        for idx, (s, kt) in enumerate(kts):
                            nc.tensor.matmul(pg[:, j, :], pt[:, s, h, :],
                                             vtok[:, kt, :],
                                             start=(idx == 0),
                                             stop=(idx == len(kts) - 1))
                    nc.vector.tensor_add(rec8[:, g0:g0 + 4],
                                         pg[:, :, HD:HD + 1],
                                         esink_sb[:, g0:g0 + 4])
                    nc.vector.reciprocal(rec8[:, g0:g0 + 4], rec8[:, g0:g0 + 4])
                    rec3 = bass.AP(tensor=rec8.tensor,
                                   offset=rec8[:, g0:g0 + 4].offset,
                                   ap=[rec8.ap[0], [1, 4], [0, HD]])
                    nc.vector.tensor_mul(an[:, g0:g0 + 4, :], pg[:, :, 0:HD],
                                         rec3)
                # transpose to out-proj lhsT layout [128 hd, 128 tok]
                for kk in range(4):
                    pat = pG.tile([P, P], dt.bfloat16, tag="pg", name="pat")
                    nc.tensor.transpose(pat, a2[:, kk * P:(kk + 1) * P],
                                        identb)
                    if kk % 2 == 0:
                        nc.vector.tensor_copy(att[:, kk, :], pat)
                    else:
                        nc.scalar.copy(att[:, kk, :], pat)
                return att

            # ---- out projection; one y DMA per tile (finer on the last) ----
            def out_proj(b, att):
                ysb = ysbp.tile([P, 6, YC], dt.bfloat16, tag="ysb", name="ysb")
                for ch in range(6):
                    o0 = ch * YC
                    pyt = pY.tile([P, 512], dt.float32, tag="py", name="py")
                    for kk in range(4):
                        nc.tensor.matmul(pyt[:, 0:YC], att[:, kk, :],
                                         wout_sb[kk][:, o0:o0 + YC],
                                         start=(kk == 0), stop=(kk == 3))
                    if ch % 2 == 0:
                        nc.vector.tensor_copy(ysb[:, ch, :], pyt[:, 0:YC])
                    else:
                        nc.scalar.copy(ysb[:, ch, :], pyt[:, 0:YC])
                    if b == MT - 1:
                        for q in range(2):
                            eng = nc.sync if (ch + q) % 2 == 0 else nc.scalar
                            q0 = o0 + q * (YC // 2)
                            eng.dma_start(
                                out=y_d[b * P:(b + 1) * P, q0:q0 + YC // 2],
                                in_=ysb[:, ch, q * (YC // 2):(q + 1) * (YC // 2)])
                if b < MT - 1:
                    (dma if b % 2 == 0 else dma2)(y_d[b * P:(b + 1) * P, :], ysb)

            # ---- schedule ----
            ssq_phase(0)
            ssq_phase(1)
            # both Lns, then both Exps: each activation-table set loads
            # exactly once, early; attention then stays on the exp set.
            ln_phase(0)
            ln_phase(1)
            rsq_phase(0)
            rsq_phase(1)
            kv_phase(0)
            q_phase(0)
            atts = {}
            atts[0] = attn_front(0)
            atts[1] = attn_front(1)
            out_proj(0, atts.pop(0))
            atts[2] = attn_front(2)
            out_proj(1, atts.pop(1))
            atts[3] = attn_front(3)
            out_proj(2, atts.pop(2))
            kv_phase(1)
            q_phase(1)
            atts[4] = attn_front(4)
            out_proj(3, atts.pop(3))
            for b in range(5, MT + 1):
                if b <= MT - 1:
                    atts[b] = attn_front(b)
                out_proj(b - 1, atts.pop(b - 1))

    nc.compile()
    return nc


# ----------------------------------------------------------------------------
# public entry
# ----------------------------------------------------------------------------

LAST_RESULTS = None


def kernel(x, norm_scale, qkv_w, qkv_b, out_w, out_b, sinks):
    global LAST_RESULTS
    x = np.asarray(x, dtype=np.float32)
    norm_scale = np.asarray(norm_scale, dtype=np.float32)
    qkv_w = np.asarray(qkv_w, dtype=np.float32)
    qkv_b = np.asarray(qkv_b, dtype=np.float32)
    out_w = np.asarray(out_w, dtype=np.float32)
    out_b = np.asarray(out_b, dtype=np.float32)
    sinks = np.asarray(sinks, dtype=np.float32)

    if "nc" not in _CACHE:
        _CACHE["nc"] = build_nc()
    nc = _CACHE["nc"]

    in_maps = [
        _prep_core_inputs(c, x, norm_scale, qkv_w, qkv_b, out_w, sinks)
        for c in range(NCORES)
    ]
    import os
    tmpdir = os.environ.get("BASS_TMPDIR") or None
    res = run_bass_kernel_spmd(nc, in_maps, core_ids=list(range(NCORES)),
                               tmpdir=tmpdir)
    LAST_RESULTS = res
    y = np.zeros((T, HIDDEN), dtype=np.float64)
    for c in range(NCORES):
        y += res.results[c]["y"].astype(np.float64)
    y += out_b.astype(np.float64)[None, :]
    return y.astype(np.float32)


# revision 32
# speedup vs baseline: 1.2687x; 1.0198x over previous
"""Trainium2 Bass kernel: sparse (sliding-window) attention block, v4.

Full module per reference:
  RMSNorm -> fused QKV (5120x2880) -> YaRN RoPE -> GQA sliding-window(128)
  causal attention with learned sink logit -> out projection (2880x4096).

Sharding: tensor-parallel over heads across 8 cores. Core c owns q-heads
[8c, 8c+8) and kv-head c. Each core emits a partial [1024, 2880] bf16
output; the host sums the partials (f64) and adds out_b.

v4 changes over v3 (all bf16 GEMMs kept -- fp8 matmuls were measured to
add ~8e-2 rel error through the logits, far over budget):
  - sum-of-squares via host-precomputed x^2 in fp8e4 + PE DoubleRow
    ones-reduction (2.6us PE instead of 9.8us PE + 10.8us DVE).
  - qkv bias folded into the matmul: xt row 2880 holds rms = 1/rsq
    (written on device), wq column k=2880 holds qkv_b, so
    psum = W x + b/rsq and one DVE mul by rsq finishes the tile.
  - rope tables in bf16 (DVE 2x mode) and indexed at the destination
    partition, so no separate negated-sin table is needed.
  - all rope sign-muls and mask muls on DVE (gpsimd ops measured
    1.1-1.7us each and serialized the attention pipeline in v3).
  - attention split into attn_front(b) (scores..att tile) and
    out_proj(b); out_proj(b-1) is emitted after attn_front(b) so its
    24 matmuls fill the exp/mask latency gaps on the PE.
"""

import math
import sys

import numpy as np

try:
    import concourse.bass as bass
except ImportError:  # pragma: no cover
    sys.path.insert(0, "/opt/trn_rl_repo")
    import concourse.bass as bass

import concourse.bacc as bacc
import concourse.tile as tile
from concourse import mybir
from concourse.masks import make_identity
from concourse.bass_utils import run_bass_kernel_spmd

import ml_dtypes

BF16 = ml_dtypes.bfloat16
F8 = ml_dtypes.float8_e4m3

T = 1024
HIDDEN = 2880
HD = 64
NH = 64
NKV = 8
SW = 128
NCORES = 8
HPC = NH // NCORES          # q heads per core = 8
QKV_DIM = HD * (NH + 2 * NKV)
SM_SCALE = 1.0 / math.sqrt(HD)

P = 128
KT = (HIDDEN + P - 1) // P   # 23 k-tiles over hidden (zero-padded to 2944)
KPAD = KT * P
KSQ = 24                     # xsq k-tiles (padded to an even count for DoubleRow)
NT = 5                       # qkv n-tiles of 128 (4 q-tiles + 1 kv-tile)
MT = T // P                  # 8 token tiles
AW = HD + 1                  # AV width: 64 v dims + denominator column
BIAS_K = HIDDEN              # contraction index carrying the bias (ki=22, kp=64)

dt = mybir.dt

_CACHE = {}


# ----------------------------------------------------------------------------
# host-side helpers
# ----------------------------------------------------------------------------

def _rope_cos_sin(num_tokens):
    base = 150000.0
    scaling = 32.0
    init_ctx = 4096.0
    ntk_alpha = 1.0
    ntk_beta = 32.0
    d_half = HD / 2
    freq = base ** (np.arange(0, HD, 2, dtype=np.float32) / HD)
    concentration = 0.1 * math.log(scaling) + 1.0
    low = d_half * math.log(init_ctx / (ntk_beta * 2 * math.pi)) / math.log(base)
    high = d_half * math.log(init_ctx / (ntk_alpha * 2 * math.pi)) / math.log(base)
    interpolation = 1.0 / (scaling * freq)
    extrapolation = 1.0 / freq
    ramp = (np.arange(int(d_half), dtype=np.float32) - low) / (high - low)
    m = 1.0 - np.clip(ramp, 0.0, 1.0)
    inv_freq = interpolation * (1.0 - m) + extrapolation * m
    t = np.arange(num_tokens, dtype=np.float32)
    freqs = t[:, None] * inv_freq[None, :]
    cos = (np.cos(freqs) * concentration).astype(np.float32)
    sin = (np.sin(freqs) * concentration).astype(np.float32)
    return cos, sin  # [T, 32]


def _host_tables():
    """Replicated rope tables, bf16, indexed at the DESTINATION partition:
    rope(u)[p] = u[p]*cos_t[p] + u[p^32]*sin_t[p]."""
    cos, sin = _rope_cos_sin(T)  # [1024, 32]
    sgn = np.repeat([-1.0, 1.0], 32)[:, None].astype(np.float32)
    sgn = np.tile(sgn, (2, 1))  # [128, 1]
    cos_t = np.tile(cos.T, (4, 1)).astype(BF16)          # [128, 1024]
    sin_t = (np.tile(sin.T, (4, 1)) * sgn).astype(BF16)  # [128, 1024]
    # walrus requires both SBUF inputs of a TensorTensor op to share the
    # base partition, so the sin table is indexed at the SOURCE partition:
    # ts[p] = u[p^32]*sin_t[p] = u[p^32]*sneg_t[p^32], sneg_t = -sin_t.
    return cos_t, (-sin_t.astype(np.float32)).astype(BF16)


def _host_masks01():
    j = np.arange(P)[:, None]   # kt row (partition)
    i = np.arange(P)[None, :]   # q col (free)
    mask_prev = (j > i).astype(np.float32)    # dist in [1,127]
    mask_self = (j <= i).astype(np.float32)   # dist in [0,127]
    return np.concatenate([mask_prev, mask_self], axis=1).astype(BF16)


def _prep_core_inputs(core, x, norm_scale, qkv_w, qkv_b, out_w, sinks):
    q_end = NH * HD
    k_end = q_end + NKV * HD

    qrows = np.arange(core * HPC * HD, (core + 1) * HPC * HD)
    krows = np.arange(q_end + core * HD, q_end + (core + 1) * HD)
    vrows = np.arange(k_end + core * HD, k_end + (core + 1) * HD)
    # kv n-tile: v in partitions 0:64, k in 64:128
    rows = np.concatenate([qrows, vrows, krows])  # [640]

    wshard = (qkv_w[rows, :] * norm_scale[None, :]).astype(np.float32)
    bshard = qkv_b[rows].astype(np.float32)  # [640]

    # lhsT tiles: wq[n, kp, ki*128 + nc] = wshard[n*128 + nc, ki*128 + kp];
    # the bias rides contraction index k=2880 (ki=22, kp=64) whose xt row is
    # filled with rms = 1/rsq on device.
    wq = np.zeros((NT, P, KPAD), dtype=BF16)
    for n in range(NT):
        blk = wshard[n * P:(n + 1) * P, :]  # [128 n, 2880 k]
        for ki in range(KT):
            k0 = ki * P
            ksz = min(P, HIDDEN - k0)
            wq[n, :ksz, ki * P:ki * P + P] = blk[:, k0:k0 + ksz].T.astype(BF16)
        wq[n, BIAS_K - 22 * P, 22 * P:23 * P] = bshard[n * P:(n + 1) * P].astype(BF16)

    cols = np.arange(core * HPC * HD, (core + 1) * HPC * HD)
    wo = out_w[:, cols].T.astype(np.float32)  # [512 hd, 2880 H]
    wout = wo.reshape(4, P, HIDDEN).astype(BF16)

    cos_t, sin_t = _host_tables()  # [128, 1024] bf16

    xt = np.zeros((KPAD, T), dtype=BF16)
    xt[:HIDDEN] = x.T.astype(BF16)

    xsq = np.zeros((KSQ * P, T), dtype=F8)
    xsq[:HIDDEN] = np.clip(x.T.astype(np.float64) ** 2, 0, 448).astype(F8)

    esink = np.exp(sinks[core * HPC:(core + 1) * HPC].astype(np.float64))
    esink = np.broadcast_to(esink.astype(np.float32), (P, HPC)).copy()

    return {
        "xt": xt,                               # [2944, 1024] bf16
        "xsq": xsq,                             # [3072, 1024] fp8e4
        "wq": wq,                               # [5, 128, 2944] bf16
        "wout": wout,                           # [4, 128, 2880] bf16
        "cos_t": cos_t, "sin_t": sin_t,         # [128, 1024] bf16
        "mask": _host_masks01(),                # [128, 256] bf16
        "esink": esink,                         # [128, 8] f32
    }


# ----------------------------------------------------------------------------
# device kernel (Tile)
# ----------------------------------------------------------------------------

def build_nc():
    nc = bacc.Bacc("TRN2", target_bir_lowering=False, debug=False)

    xt_d = nc.dram_tensor("xt", [KPAD, T], dt.bfloat16, kind="ExternalInput").ap()
    xsq_d = nc.dram_tensor("xsq", [KSQ * P, T], dt.float8e4,
                           kind="ExternalInput").ap()
    wq_d = nc.dram_tensor("wq", [NT, P, KPAD], dt.bfloat16, kind="ExternalInput").ap()
    wout_d = nc.dram_tensor("wout", [4, P, HIDDEN], dt.bfloat16,
                            kind="ExternalInput").ap()
    cos_d = nc.dram_tensor("cos_t", [P, T], dt.bfloat16, kind="ExternalInput").ap()
    sin_d = nc.dram_tensor("sin_t", [P, T], dt.bfloat16, kind="ExternalInput").ap()
    mask_d = nc.dram_tensor("mask", [P, 2 * P], dt.bfloat16, kind="ExternalInput").ap()
    esink_d = nc.dram_tensor("esink", [P, HPC], dt.float32, kind="ExternalInput").ap()
    y_d = nc.dram_tensor("y", [T, HIDDEN], dt.bfloat16, kind="ExternalOutput").ap()

    YC = 480

    def bcast_mid(ap2d, n):
        """[P, F] -> [P, n, F] with a 0-step middle dim (free broadcast)."""
        return bass.AP(tensor=ap2d.tensor, offset=ap2d.offset,
                       ap=[ap2d.ap[0], [0, n]] + list(ap2d.ap[1:]))

    def dram_tiles(d_ap, r0, nt, c0, csz):
        """[nt*128 rows from r0, cols c0:c0+csz] of a 2D dram tensor as a
        [128, nt, csz] AP (partition-major k-tiles)."""
        rstride = d_ap.ap[0][0]
        return bass.AP(tensor=d_ap.tensor, offset=d_ap.offset + r0 * rstride + c0,
                       ap=[[rstride, P], [P * rstride, nt], [1, csz]])

    with tile.TileContext(nc) as tc:
        with (
            tc.tile_pool(name="const", bufs=1) as const,
            tc.tile_pool(name="res", bufs=1) as res,
            tc.tile_pool(name="xsqp", bufs=2) as xsqp,
            tc.tile_pool(name="qkvp", bufs=2) as qkvp,
            tc.tile_pool(name="ropep", bufs=2) as ropep,
            tc.tile_pool(name="ptp", bufs=2) as ptp,
            tc.tile_pool(name="smallp", bufs=3) as smallp,
            tc.tile_pool(name="anp", bufs=2) as anp,
            tc.tile_pool(name="attp", bufs=2) as attp,
            tc.tile_pool(name="ysbp", bufs=3) as ysbp,
            tc.tile_pool(name="pA", bufs=2, space="PSUM") as pA,
            tc.tile_pool(name="pY", bufs=2, space="PSUM") as pY,
            tc.tile_pool(name="pS", bufs=2, space="PSUM") as pS,
            tc.tile_pool(name="pG", bufs=2, space="PSUM") as pG,
        ):
            # ---- constants ----
            zbias = const.tile([P, 1], dt.float32, tag="zbias", name="zbias")
            nc.vector.memset(zbias, 0.0)
            eps_t = const.tile([1, 1], dt.float32, tag="eps", name="eps_t")
            nc.vector.memset(eps_t, 1e-5)
            eps128 = const.tile([P, 1], dt.float32, tag="eps128", name="eps128")
            nc.vector.memset(eps128, 1e-5)
            # all-ones stationary [K, 2, 128]: every psum partition gets ssq,
            # so no partition_broadcast is needed for rsq.
            ones8 = const.tile([P, 2, P], dt.float8e4, tag="ones8", name="ones8")
            nc.vector.memset(ones8, 1.0)
            identb = const.tile([P, P], dt.bfloat16, tag="identb", name="identb")
            make_identity(nc, identb)
            dmy = const.tile([1, 2], dt.float32, tag="dmy", name="dmy")

            # ---- persistent SBUF ----
            wqall = res.tile([P, 4, KPAD], dt.bfloat16, tag="wqall", name="wqall")
            wq4t = res.tile([P, KPAD], dt.bfloat16, tag="wq4", name="wq4")
            wq_sb = [wqall[:, n, :] for n in range(4)] + [wq4t]
            xt_sb = res.tile([P, KT, T], dt.bfloat16, tag="xt", name="xt")
            woutall = res.tile([P, 4, HIDDEN], dt.bfloat16, tag="woutall",
                               name="woutall")
            wout_sb = [woutall[:, kk, :] for kk in range(4)]
            cos_sb = const.tile([P, T], dt.bfloat16, tag="cos", name="cos_sb")
            sin_sb = const.tile([P, T], dt.bfloat16, tag="sin", name="sin_sb")
            mask_sb = const.tile([P, 2 * P], dt.bfloat16, tag="mask", name="mask_sb")
            esink_sb = const.tile([P, HPC], dt.float32, tag="esink", name="esink_sb")

            lnm = res.tile([P, T], dt.float32, tag="lnm", name="lnm")
            rsq_b = res.tile([P, T], dt.float32, tag="rsq", name="rsq_b")

            qra = res.tile([64, HPC, T], dt.bfloat16, tag="qra", name="qra")
            krope = res.tile([64, T], dt.bfloat16, tag="krope", name="krope")
            vtok = res.tile([P, MT, AW], dt.bfloat16, tag="vtok", name="vtok")
            nc.vector.memset(vtok[:, :, HD:HD + 1], 1.0)
            qkvT4 = res.tile([P, T], dt.bfloat16, tag="qkvT4", name="qkvT4")

            def dma(out, in_):
                nc.sync.dma_start(out=out, in_=in_)

            def dma2(out, in_):
                nc.scalar.dma_start(out=out, in_=in_)

            # ---- DMA issue ----
            # All bulk loads ride the sync queue; the scalar queue stays free
            # for the rsq activations and attention work (DMA-ring semaphore
            # waits on a clogged queue were measured delaying the queued
            # activations by 35us).
            xsq_sb = []
            for half in range(2):
                xs = xsqp.tile([P, KSQ, 512], dt.float8e4, tag="xsq",
                               name=f"xsq{half}")
                dma(xs, dram_tiles(xsq_d, 0, KSQ, half * 512, 512))
                xsq_sb.append(xs)
            dma2(cos_sb, cos_d)
            dma2(sin_sb, sin_d)
            dma2(mask_sb, mask_d)
            dma2(esink_sb, esink_d)
            dma(wq4t, wq_d[4])

            def wq_pair(n0):
                dma(wqall[:, n0:n0 + 2, :],
                    bass.AP(tensor=wq_d.tensor,
                            offset=wq_d.offset + n0 * P * KPAD,
                            ap=[[KPAD, P], [P * KPAD, 2], [1, KPAD]]))

            def xt_chunk(i, half):
                t0, t1 = 8 * i, min(8 * (i + 1), KT)
                dma(xt_sb[:, t0:t1, half * 512:half * 512 + 512],
                    dram_tiles(xt_d, t0 * P, t1 - t0, half * 512, 512))

            xt_chunk(0, 0)
            xt_chunk(1, 0)
            wq_pair(0)
            xt_chunk(2, 0)
            wq_pair(2)
            dma(woutall, bass.AP(tensor=wout_d.tensor, offset=wout_d.offset,
                                 ap=[[HIDDEN, P], [P * HIDDEN, 4], [1, HIDDEN]]))
            for i in range(3):  # xt half 1
                xt_chunk(i, 1)

            def rsq_full():
                # one full-width Exp after both Lns: [Ln, Ln, Exp] keeps each
                # activation-table set loaded exactly once, before attention
                nc.scalar.activation(rsq_b, lnm,
                                     mybir.ActivationFunctionType.Exp,
                                     bias=zbias, scale=-0.5)
                # bias rows: xt[k=2880] = rms = 1/rsq so psum = Wx + b/rsq.
                # Per half so kv0's k-tile 22 read only depends on half 0.
                with nc.allow_low_precision(reason="rms bias row, bf16 ok"):
                    for half in range(2):
                        sl = slice(half * 512, half * 512 + 512)
                        nc.vector.reciprocal(xt_sb[64:65, 22, sl],
                                             rsq_b[64:65, sl])



            # ---- ssq: fp8 DoubleRow ones-reduction, per token half ----
            psum_ssq = [None, None]

            def ssq_phase(half):
                ps = pY.tile([P, 512], dt.float32, tag="py", name=f"ssq{half}")
                xs = xsq_sb[half]
                for kk in range(KSQ // 2):
                    nc.tensor.matmul(
                        ps, ones8, xs[:, 2 * kk:2 * kk + 2, :],
                        start=(kk == 0), stop=(kk == KSQ // 2 - 1),
                        perf_mode=mybir.MatmulPerfMode.DoubleRow)
                psum_ssq[half] = ps

            def ln_phase(half):
                sl = slice(half * 512, half * 512 + 512)
                nc.scalar.activation(lnm[:, sl], psum_ssq[half],
                                     mybir.ActivationFunctionType.Ln,
                                     bias=eps128, scale=1.0 / HIDDEN)

            # ---- qkv projection ----
            def qkv_tile(n, half, dst):
                """dst <- (W x + b/rsq)*rsq for columns [half*512, ...+512)."""
                c0 = half * 512
                pq = pA.tile([P, 512], dt.float32, tag="pb", name="pb")
                for ki in range(KT):
                    nc.tensor.matmul(
                        pq,
                        wq_sb[n][:, ki * P:ki * P + P],
                        xt_sb[:, ki, c0:c0 + 512],
                        start=(ki == 0), stop=(ki == KT - 1),
                    )
                nc.vector.tensor_mul(dst, pq, rsq_b[:, c0:c0 + 512])

            def kv_phase(half):
                c0 = half * 512
                sl = slice(c0, c0 + 512)
                qkv_tile(4, half, qkvT4[:, sl])
                for j in range(4):
                    b = half * 4 + j
                    pv = pG.tile([P, HD], dt.bfloat16, tag="pg", name="pv")
                    nc.tensor.transpose(pv, qkvT4[0:64, b * P:(b + 1) * P],
                                        identb[:64, :64])
                    nc.vector.tensor_copy(vtok[:, b, 0:HD], pv)
                # k rope (rows 64:128); sin table indexed at dst partition
                ktc = ropep.tile([P, 512], dt.bfloat16, tag="htc", name="ktc")
                nc.vector.tensor_mul(ktc[64:128, :], qkvT4[64:128, sl],
                                     cos_sb[64:128, sl])
                kts = ropep.tile([P, 512], dt.bfloat16, tag="hts", name="kts")
                for a in (64, 96):
                    b_ = a ^ 32
                    nc.vector.tensor_mul(kts[a:a + 32, :],
                                         qkvT4[b_:b_ + 32, sl],
                                         sin_sb[b_:b_ + 32, sl])
                nc.vector.tensor_add(krope[:, sl], ktc[64:128, :],
                                     kts[64:128, :])

            def q_phase(half, ns=range(4)):
                c0 = half * 512
                sl = slice(c0, c0 + 512)
                for n in ns:
                    qkvt = qkvp.tile([P, 512], dt.bfloat16, tag="qkvT",
                                     name=f"qkvt{n}")
                    qkv_tile(n, half, qkvt)
                    tc_ = ropep.tile([P, 512], dt.bfloat16, tag="htc", name="htc")
                    nc.vector.tensor_mul(tc_, qkvt, cos_sb[:, sl])
                    ts_ = ropep.tile([P, 512], dt.bfloat16, tag="hts", name="hts")
                    for a in range(0, P, 32):
                        b_ = a ^ 32
                        nc.vector.tensor_mul(ts_[a:a + 32, :],
                                             qkvt[b_:b_ + 32, :],
                                             sin_sb[b_:b_ + 32, sl])
                    for i in range(2):
                        b0 = 64 * i
                        nc.vector.tensor_add(qra[:, 2 * n + i, sl],
                                             tc_[b0:b0 + 64, :],
                                             ts_[b0:b0 + 64, :])

            # ---- attention, split so out_proj(b-1) can fill the exp/mask
            # latency between scores(b) and av(b) on the PE queue ----
            def scores_phase(b):
                pt = ptp.tile([P, 2, HPC, P], dt.bfloat16, tag="pt", name="pt")
                kts = [(0, b - 1), (1, b)] if b > 0 else [(1, b)]
                for g in range(2):
                    g0 = 4 * g
                    for s, kt in kts:
                        ps = pS.tile([P, 4, P], dt.float32, tag="sc", name="sc")
                        nc.tensor.matmul(
                            ps,
                            krope[:, kt * P:(kt + 1) * P],
                            qra[:, g0:g0 + 4, b * P:(b + 1) * P],
                            start=True, stop=True)
                        nc.scalar.activation(pt[:, s, g0:g0 + 4, :], ps,
                                             mybir.ActivationFunctionType.Exp,
                                             bias=zbias, scale=SM_SCALE)
                        m0 = 0 if s == 0 else P
                        nc.vector.tensor_mul(pt[:, s, g0:g0 + 4, :],
                                             pt[:, s, g0:g0 + 4, :],
                                             bcast_mid(mask_sb[:, m0:m0 + P], 4))
                return pt, kts

            def av_phase(b, st):
                pt, kts = st
                rec8 = smallp.tile([P, HPC], dt.float32, tag="rec8", name="rec8")
                an = anp.tile([P, HPC, HD], dt.bfloat16, tag="an", name="an")
                att = attp.tile([P, 4, P], dt.bfloat16, tag="att", name="att")
                a2 = an.rearrange("p a b -> p (a b)")
                for g in range(2):
                    g0 = 4 * g
                    # AV with ones column -> denominators in column 64
                    pg = pG.tile([P, 4, AW], dt.float32, tag="pg", name="pg")
                    for j in range(4):
                        h = g0 + j
                        for idx, (s, kt) in enumerate(kts):
                            nc.tensor.matmul(pg[:, j, :], pt[:, s, h, :],
                                             vtok[:, kt, :],
                                             start=(idx == 0),
                                             stop=(idx == len(kts) - 1))
                    nc.vector.tensor_add(rec8[:, g0:g0 + 4],
                                         pg[:, :, HD:HD + 1],
                                         esink_sb[:, g0:g0 + 4])
                    nc.vector.reciprocal(rec8[:, g0:g0 + 4], rec8[:, g0:g0 + 4])
                    rec3 = bass.AP(tensor=rec8.tensor,
                                   offset=rec8[:, g0:g0 + 4].offset,
                                   ap=[rec8.ap[0], [1, 4], [0, HD]])
                    nc.vector.tensor_mul(an[:, g0:g0 + 4, :], pg[:, :, 0:HD],
                                         rec3)
                # transpose to out-proj lhsT layout [128 hd, 128 tok]
                for kk in range(4):
                    pat = pG.tile([P, P], dt.bfloat16, tag="pg", name="pat")
                    nc.tensor.transpose(pat, a2[:, kk * P:(kk + 1) * P],
                                        identb)
                    if kk % 2 == 0:
                        nc.vector.tensor_copy(att[:, kk, :], pat)
                    else:
                        nc.scalar.copy(att[:, kk, :], pat)
                return att

            # ---- out projection; one y DMA per tile (finer on the last) ----
            def out_proj(b, att):
                ysb = ysbp.tile([P, 6, YC], dt.bfloat16, tag="ysb", name="ysb")
                for ch in range(6):
                    o0 = ch * YC
                    pyt = pY.tile([P, 512], dt.float32, tag="py", name="py")
                    for kk in range(4):
                        nc.tensor.matmul(pyt[:, 0:YC], att[:, kk, :],
                                         wout_sb[kk][:, o0:o0 + YC],
                                         start=(kk == 0), stop=(kk == 3))
                    if ch % 2 == 0:
                        nc.vector.tensor_copy(ysb[:, ch, :], pyt[:, 0:YC])
                    else:
                        nc.scalar.copy(ysb[:, ch, :], pyt[:, 0:YC])
                    if b == MT - 1:
                        for q in range(2):
                            eng = nc.sync if (ch + q) % 2 == 0 else nc.scalar
                            q0 = o0 + q * (YC // 2)
                            eng.dma_start(
                                out=y_d[b * P:(b + 1) * P, q0:q0 + YC // 2],
                                in_=ysb[:, ch, q * (YC // 2):(q + 1) * (YC // 2)])
                if b < MT - 1:
                    (dma if b % 2 == 0 else dma2)(y_d[b * P:(b + 1) * P, :], ysb)

            # ---- schedule ----
            ssq_phase(0)
            ln_phase(0)
            ssq_phase(1)
            ln_phase(1)
            rsq_full()
            kv_phase(0)
            q_phase(0)
            sc = {}
            att_t = {}
            sc[0] = scores_phase(0)
            sc[1] = scores_phase(1)
            att_t[0] = av_phase(0, sc.pop(0))
            sc[2] = scores_phase(2)
            out_proj(0, att_t.pop(0))
            att_t[1] = av_phase(1, sc.pop(1))
            sc[3] = scores_phase(3)
            out_proj(1, att_t.pop(1))
            att_t[2] = av_phase(2, sc.pop(2))
            kv_phase(1)
            out_proj(2, att_t.pop(2))
            att_t[3] = av_phase(3, sc.pop(3))
            q_phase(1)
            sc[4] = scores_phase(4)
            out_proj(3, att_t.pop(3))
            att_t[4] = av_phase(4, sc.pop(4))
            for b in range(5, MT):
                sc[b] = scores_phase(b)
                out_proj(b - 1, att_t.pop(b - 1))
                att_t[b] = av_phase(b, sc.pop(b))
            out_proj(MT - 1, att_t.pop(MT - 1))

    nc.compile()
    return nc


# ----------------------------------------------------------------------------
# public entry
# ----------------------------------------------------------------------------

LAST_RESULTS = None


def kernel(x, norm_scale, qkv_w, qkv_b, out_w, out_b, sinks):
    global LAST_RESULTS
    x = np.asarray(x, dtype=np.float32)
    norm_scale = np.asarray(norm_scale, dtype=np.float32)
    qkv_w = np.asarray(qkv_w, dtype=np.float32)
    qkv_b = np.asarray(qkv_b, dtype=np.float32)
    out_w = np.asarray(out_w, dtype=np.float32)
    out_b = np.asarray(out_b, dtype=np.float32)
    sinks = np.asarray(sinks, dtype=np.float32)

    if "nc" not in _CACHE:
        _CACHE["nc"] = build_nc()
    nc = _CACHE["nc"]

    in_maps = [
        _prep_core_inputs(c, x, norm_scale, qkv_w, qkv_b, out_w, sinks)
        for c in range(NCORES)
    ]
    import os
    tmpdir = os.environ.get("BASS_TMPDIR") or None
    res = run_bass_kernel_spmd(nc, in_maps, core_ids=list(range(NCORES)),
                               tmpdir=tmpdir)
    LAST_RESULTS = res
    y = np.zeros((T, HIDDEN), dtype=np.float64)
    for c in range(NCORES):
        y += res.results[c]["y"].astype(np.float64)
    y += out_b.astype(np.float64)[None, :]
    return y.astype(np.float32)


# revision 35
# speedup vs baseline: 1.3409x; 1.0569x over previous
"""Trainium2 Bass kernel: sparse (sliding-window) attention block, v4.

Full module per reference:
  RMSNorm -> fused QKV (5120x2880) -> YaRN RoPE -> GQA sliding-window(128)
  causal attention with learned sink logit -> out projection (2880x4096).

Sharding: tensor-parallel over heads across 8 cores. Core c owns q-heads
[8c, 8c+8) and kv-head c. Each core emits a partial [1024, 2880] bf16
output; the host sums the partials (f64) and adds out_b.

v4 changes over v3 (all bf16 GEMMs kept -- fp8 matmuls were measured to
add ~8e-2 rel error through the logits, far over budget):
  - sum-of-squares via host-precomputed x^2 in fp8e4 + PE DoubleRow
    ones-reduction (2.6us PE instead of 9.8us PE + 10.8us DVE).
  - qkv bias folded into the matmul: xt row 2880 holds rms = 1/rsq
    (written on device), wq column k=2880 holds qkv_b, so
    psum = W x + b/rsq and one DVE mul by rsq finishes the tile.
  - rope tables in bf16 (DVE 2x mode) and indexed at the destination
    partition, so no separate negated-sin table is needed.
  - all rope sign-muls and mask muls on DVE (gpsimd ops measured
    1.1-1.7us each and serialized the attention pipeline in v3).
  - attention split into attn_front(b) (scores..att tile) and
    out_proj(b); out_proj(b-1) is emitted after attn_front(b) so its
    24 matmuls fill the exp/mask latency gaps on the PE.
"""

import math
import sys

import numpy as np

try:
    import concourse.bass as bass
except ImportError:  # pragma: no cover
    sys.path.insert(0, "/opt/trn_rl_repo")
    import concourse.bass as bass

import concourse.bacc as bacc
import concourse.tile as tile
from concourse import mybir
from concourse.masks import make_identity
from concourse.bass_utils import run_bass_kernel_spmd

import ml_dtypes

BF16 = ml_dtypes.bfloat16
F8 = ml_dtypes.float8_e4m3

T = 1024
HIDDEN = 2880
HD = 64
NH = 64
NKV = 8
SW = 128
NCORES = 8
HPC = NH // NCORES          # q heads per core = 8
QKV_DIM = HD * (NH + 2 * NKV)
SM_SCALE = 1.0 / math.sqrt(HD)

P = 128
KT = (HIDDEN + P - 1) // P   # 23 k-tiles over hidden (zero-padded to 2944)
KPAD = KT * P
KSQ = 24                     # xsq k-tiles (padded to an even count for DoubleRow)
NT = 5                       # qkv n-tiles of 128 (4 q-tiles + 1 kv-tile)
MT = T // P                  # 8 token tiles
AW = HD + 1                  # AV width: 64 v dims + denominator column
BIAS_K = HIDDEN              # contraction index carrying the bias (ki=22, kp=64)

dt = mybir.dt

_CACHE = {}


# ----------------------------------------------------------------------------
# host-side helpers
# ----------------------------------------------------------------------------

def _rope_cos_sin(num_tokens):
    base = 150000.0
    scaling = 32.0
    init_ctx = 4096.0
    ntk_alpha = 1.0
    ntk_beta = 32.0
    d_half = HD / 2
    freq = base ** (np.arange(0, HD, 2, dtype=np.float32) / HD)
    concentration = 0.1 * math.log(scaling) + 1.0
    low = d_half * math.log(init_ctx / (ntk_beta * 2 * math.pi)) / math.log(base)
    high = d_half * math.log(init_ctx / (ntk_alpha * 2 * math.pi)) / math.log(base)
    interpolation = 1.0 / (scaling * freq)
    extrapolation = 1.0 / freq
    ramp = (np.arange(int(d_half), dtype=np.float32) - low) / (high - low)
    m = 1.0 - np.clip(ramp, 0.0, 1.0)
    inv_freq = interpolation * (1.0 - m) + extrapolation * m
    t = np.arange(num_tokens, dtype=np.float32)
    freqs = t[:, None] * inv_freq[None, :]
    cos = (np.cos(freqs) * concentration).astype(np.float32)
    sin = (np.sin(freqs) * concentration).astype(np.float32)
    return cos, sin  # [T, 32]


def _host_tables():
    """Replicated rope tables, bf16, indexed at the DESTINATION partition:
    rope(u)[p] = u[p]*cos_t[p] + u[p^32]*sin_t[p]."""
    cos, sin = _rope_cos_sin(T)  # [1024, 32]
    sgn = np.repeat([-1.0, 1.0], 32)[:, None].astype(np.float32)
    sgn = np.tile(sgn, (2, 1))  # [128, 1]
    cos_t = np.tile(cos.T, (4, 1)).astype(BF16)          # [128, 1024]
    sin_t = (np.tile(sin.T, (4, 1)) * sgn).astype(BF16)  # [128, 1024]
    # walrus requires both SBUF inputs of a TensorTensor op to share the
    # base partition, so the sin table is indexed at the SOURCE partition:
    # ts[p] = u[p^32]*sin_t[p] = u[p^32]*sneg_t[p^32], sneg_t = -sin_t.
    return cos_t, (-sin_t.astype(np.float32)).astype(BF16)


def _host_masks01():
    j = np.arange(P)[:, None]   # kt row (partition)
    i = np.arange(P)[None, :]   # q col (free)
    mask_prev = (j > i).astype(np.float32)    # dist in [1,127]
    mask_self = (j <= i).astype(np.float32)   # dist in [0,127]
    return np.concatenate([mask_prev, mask_self], axis=1).astype(BF16)


def _prep_core_inputs(core, x, norm_scale, qkv_w, qkv_b, out_w, sinks):
    q_end = NH * HD
    k_end = q_end + NKV * HD

    qrows = np.arange(core * HPC * HD, (core + 1) * HPC * HD)
    krows = np.arange(q_end + core * HD, q_end + (core + 1) * HD)
    vrows = np.arange(k_end + core * HD, k_end + (core + 1) * HD)
    # kv n-tile: v in partitions 0:64, k in 64:128
    rows = np.concatenate([qrows, vrows, krows])  # [640]

    wshard = (qkv_w[rows, :] * norm_scale[None, :]).astype(np.float32)
    bshard = qkv_b[rows].astype(np.float32)  # [640]

    # lhsT tiles: wq[n, kp, ki*128 + nc] = wshard[n*128 + nc, ki*128 + kp];
    # the bias rides contraction index k=2880 (ki=22, kp=64) whose xt row is
    # filled with rms = 1/rsq on device.
    wq = np.zeros((NT, P, KPAD), dtype=BF16)
    for n in range(NT):
        blk = wshard[n * P:(n + 1) * P, :]  # [128 n, 2880 k]
        for ki in range(KT):
            k0 = ki * P
            ksz = min(P, HIDDEN - k0)
            wq[n, :ksz, ki * P:ki * P + P] = blk[:, k0:k0 + ksz].T.astype(BF16)
        wq[n, BIAS_K - 22 * P, 22 * P:23 * P] = bshard[n * P:(n + 1) * P].astype(BF16)

    cols = np.arange(core * HPC * HD, (core + 1) * HPC * HD)
    wo = out_w[:, cols].T.astype(np.float32)  # [512 hd, 2880 H]
    wout = wo.reshape(4, P, HIDDEN).astype(BF16)

    cos_t, sin_t = _host_tables()  # [128, 1024] bf16

    xt = np.zeros((KPAD, T), dtype=BF16)
    xt[:HIDDEN] = x.T.astype(BF16)

    xsq = np.zeros((KSQ * P, T), dtype=F8)
    xsq[:HIDDEN] = np.clip(x.T.astype(np.float64) ** 2, 0, 448).astype(F8)

    esink = np.exp(sinks[core * HPC:(core + 1) * HPC].astype(np.float64))
    esink = np.broadcast_to(esink.astype(np.float32), (P, HPC)).copy()

    return {
        "xt": xt,                               # [2944, 1024] bf16
        "xsq": xsq,                             # [3072, 1024] fp8e4
        "wq": wq,                               # [5, 128, 2944] bf16
        "wout": wout,                           # [4, 128, 2880] bf16
        "cos_t": cos_t, "sin_t": sin_t,         # [128, 1024] bf16
        "mask": _host_masks01(),                # [128, 256] bf16
        "esink": esink,                         # [128, 8] f32
    }


# ----------------------------------------------------------------------------
# device kernel (Tile)
# ----------------------------------------------------------------------------

def build_nc():
    nc = bacc.Bacc("TRN2", target_bir_lowering=False, debug=False)

    xt_d = nc.dram_tensor("xt", [KPAD, T], dt.bfloat16, kind="ExternalInput").ap()
    xsq_d = nc.dram_tensor("xsq", [KSQ * P, T], dt.float8e4,
                           kind="ExternalInput").ap()
    wq_d = nc.dram_tensor("wq", [NT, P, KPAD], dt.bfloat16, kind="ExternalInput").ap()
    wout_d = nc.dram_tensor("wout", [4, P, HIDDEN], dt.bfloat16,
                            kind="ExternalInput").ap()
    cos_d = nc.dram_tensor("cos_t", [P, T], dt.bfloat16, kind="ExternalInput").ap()
    sin_d = nc.dram_tensor("sin_t", [P, T], dt.bfloat16, kind="ExternalInput").ap()
    mask_d = nc.dram_tensor("mask", [P, 2 * P], dt.bfloat16, kind="ExternalInput").ap()
    esink_d = nc.dram_tensor("esink", [P, HPC], dt.float32, kind="ExternalInput").ap()
    y_d = nc.dram_tensor("y", [T, HIDDEN], dt.bfloat16, kind="ExternalOutput").ap()

    YC = 480

    def bcast_mid(ap2d, n):
        """[P, F] -> [P, n, F] with a 0-step middle dim (free broadcast)."""
        return bass.AP(tensor=ap2d.tensor, offset=ap2d.offset,
                       ap=[ap2d.ap[0], [0, n]] + list(ap2d.ap[1:]))

    def dram_tiles(d_ap, r0, nt, c0, csz):
        """[nt*128 rows from r0, cols c0:c0+csz] of a 2D dram tensor as a
        [128, nt, csz] AP (partition-major k-tiles)."""
        rstride = d_ap.ap[0][0]
        return bass.AP(tensor=d_ap.tensor, offset=d_ap.offset + r0 * rstride + c0,
                       ap=[[rstride, P], [P * rstride, nt], [1, csz]])

    with tile.TileContext(nc) as tc:
        with (
            tc.tile_pool(name="const", bufs=1) as const,
            tc.tile_pool(name="res", bufs=1) as res,
            tc.tile_pool(name="xsqp", bufs=2) as xsqp,
            tc.tile_pool(name="qkvp", bufs=2) as qkvp,
            tc.tile_pool(name="ropep", bufs=2) as ropep,
            tc.tile_pool(name="ptp", bufs=2) as ptp,
            tc.tile_pool(name="smallp", bufs=3) as smallp,
            tc.tile_pool(name="anp", bufs=2) as anp,
            tc.tile_pool(name="attp", bufs=2) as attp,
            tc.tile_pool(name="ysbp", bufs=3) as ysbp,
            tc.tile_pool(name="pA", bufs=2, space="PSUM") as pA,
            tc.tile_pool(name="pY", bufs=2, space="PSUM") as pY,
            tc.tile_pool(name="pS", bufs=2, space="PSUM") as pS,
            tc.tile_pool(name="pG", bufs=2, space="PSUM") as pG,
        ):
            # ---- constants ----
            zbias = const.tile([P, 1], dt.float32, tag="zbias", name="zbias")
            nc.vector.memset(zbias, 0.0)
            eps_t = const.tile([1, 1], dt.float32, tag="eps", name="eps_t")
            nc.vector.memset(eps_t, 1e-5)
            eps128 = const.tile([P, 1], dt.float32, tag="eps128", name="eps128")
            nc.vector.memset(eps128, 1e-5)
            # all-ones stationary [K, 2, 128]: every psum partition gets ssq,
            # so no partition_broadcast is needed for rsq.
            ones8 = const.tile([P, 2, P], dt.float8e4, tag="ones8", name="ones8")
            nc.vector.memset(ones8, 1.0)
            identb = const.tile([P, P], dt.bfloat16, tag="identb", name="identb")
            make_identity(nc, identb)
            dmy = const.tile([1, 2], dt.float32, tag="dmy", name="dmy")

            # ---- persistent SBUF ----
            wqall = res.tile([P, 4, KPAD], dt.bfloat16, tag="wqall", name="wqall")
            wq4t = res.tile([P, KPAD], dt.bfloat16, tag="wq4", name="wq4")
            wq_sb = [wqall[:, n, :] for n in range(4)] + [wq4t]
            xt_sb = res.tile([P, KT, T], dt.bfloat16, tag="xt", name="xt")
            woutall = res.tile([P, 4, HIDDEN], dt.bfloat16, tag="woutall",
                               name="woutall")
            wout_sb = [woutall[:, kk, :] for kk in range(4)]
            cos_sb = const.tile([P, T], dt.bfloat16, tag="cos", name="cos_sb")
            sin_sb = const.tile([P, T], dt.bfloat16, tag="sin", name="sin_sb")
            mask_sb = const.tile([P, 2 * P], dt.bfloat16, tag="mask", name="mask_sb")
            esink_sb = const.tile([P, HPC], dt.float32, tag="esink", name="esink_sb")

            lnm = res.tile([P, T], dt.float32, tag="lnm", name="lnm")
            rsq_b = res.tile([P, T], dt.float32, tag="rsq", name="rsq_b")

            qra = res.tile([64, HPC, T], dt.bfloat16, tag="qra", name="qra")
            krope = res.tile([64, T], dt.bfloat16, tag="krope", name="krope")
            vtok = res.tile([P, MT, AW], dt.bfloat16, tag="vtok", name="vtok")
            nc.vector.memset(vtok[:, :, HD:HD + 1], 1.0)
            qkvT4 = res.tile([P, T], dt.bfloat16, tag="qkvT4", name="qkvT4")

            def dma(out, in_):
                nc.sync.dma_start(out=out, in_=in_)

            def dma2(out, in_):
                nc.scalar.dma_start(out=out, in_=in_)

            # ---- DMA issue ----
            # All bulk loads ride the sync queue; the scalar queue stays free
            # for the rsq activations and attention work (DMA-ring semaphore
            # waits on a clogged queue were measured delaying the queued
            # activations by 35us).
            xsq_sb = [xsqp.tile([P, KSQ, 512], dt.float8e4, tag="xsq",
                                name=f"xsq{h}") for h in range(2)]

            def wq_pair(n0):
                dma(wqall[:, n0:n0 + 2, :],
                    bass.AP(tensor=wq_d.tensor,
                            offset=wq_d.offset + n0 * P * KPAD,
                            ap=[[KPAD, P], [P * KPAD, 2], [1, KPAD]]))

            def xt_chunk(i, half):
                t0, t1 = 8 * i, min(8 * (i + 1), KT)
                dma(xt_sb[:, t0:t1, half * 512:half * 512 + 512],
                    dram_tiles(xt_d, t0 * P, t1 - t0, half * 512, 512))

            dma(xsq_sb[0], dram_tiles(xsq_d, 0, KSQ, 0, 512))
            dma2(cos_sb, cos_d)
            dma2(sin_sb, sin_d)
            dma2(mask_sb, mask_d)
            dma2(esink_sb, esink_d)
            xt_chunk(0, 0)
            xt_chunk(1, 0)
            dma(wq4t, wq_d[4])
            wq_pair(0)
            xt_chunk(2, 0)
            wq_pair(2)
            dma(xsq_sb[1], dram_tiles(xsq_d, 0, KSQ, 512, 512))
            dma(woutall, bass.AP(tensor=wout_d.tensor, offset=wout_d.offset,
                                 ap=[[HIDDEN, P], [P * HIDDEN, 4], [1, HIDDEN]]))
            for i in range(3):  # xt half 1
                xt_chunk(i, 1)



            # ---- ssq: fp8 DoubleRow ones-reduction, per token half ----
            psum_ssq = [None, None]

            def ssq_phase(half):
                ps = pY.tile([P, 512], dt.float32, tag="py", name=f"ssq{half}")
                xs = xsq_sb[half]
                for kk in range(KSQ // 2):
                    nc.tensor.matmul(
                        ps, ones8, xs[:, 2 * kk:2 * kk + 2, :],
                        start=(kk == 0), stop=(kk == KSQ // 2 - 1),
                        perf_mode=mybir.MatmulPerfMode.DoubleRow)
                psum_ssq[half] = ps

            def rsq_phase(half):
                sl = slice(half * 512, half * 512 + 512)
                nc.scalar.activation(lnm[:, sl], psum_ssq[half],
                                     mybir.ActivationFunctionType.Ln,
                                     bias=eps128, scale=1.0 / HIDDEN)
                nc.scalar.activation(rsq_b[:, sl], lnm[:, sl],
                                     mybir.ActivationFunctionType.Exp,
                                     bias=zbias, scale=-0.5)
                # bias row: xt[k=2880] = rms = 1/rsq so psum = Wx + b/rsq
                with nc.allow_low_precision(reason="rms bias row, bf16 ok"):
                    nc.vector.reciprocal(xt_sb[64:65, 22, sl],
                                         rsq_b[64:65, sl])

            # ---- qkv projection ----
            def qkv_tile(n, half, dst):
                """dst <- (W x + b/rsq)*rsq for columns [half*512, ...+512)."""
                c0 = half * 512
                pq = pA.tile([P, 512], dt.float32, tag="pb", name="pb")
                for ki in range(KT):
                    nc.tensor.matmul(
                        pq,
                        wq_sb[n][:, ki * P:ki * P + P],
                        xt_sb[:, ki, c0:c0 + 512],
                        start=(ki == 0), stop=(ki == KT - 1),
                    )
                nc.vector.tensor_mul(dst, pq, rsq_b[:, c0:c0 + 512])

            def kv_phase(half):
                c0 = half * 512
                sl = slice(c0, c0 + 512)
                qkv_tile(4, half, qkvT4[:, sl])
                for j in range(4):
                    b = half * 4 + j
                    pv = pG.tile([P, HD], dt.bfloat16, tag="pg", name="pv")
                    nc.tensor.transpose(pv, qkvT4[0:64, b * P:(b + 1) * P],
                                        identb[:64, :64])
                    nc.vector.tensor_copy(vtok[:, b, 0:HD], pv)
                # k rope (rows 64:128); sin table indexed at dst partition
                ktc = ropep.tile([P, 512], dt.bfloat16, tag="htc", name="ktc")
                nc.vector.tensor_mul(ktc[64:128, :], qkvT4[64:128, sl],
                                     cos_sb[64:128, sl])
                kts = ropep.tile([P, 512], dt.bfloat16, tag="hts", name="kts")
                for a in (64, 96):
                    b_ = a ^ 32
                    nc.vector.tensor_mul(kts[a:a + 32, :],
                                         qkvT4[b_:b_ + 32, sl],
                                         sin_sb[b_:b_ + 32, sl])
                nc.vector.tensor_add(krope[:, sl], ktc[64:128, :],
                                     kts[64:128, :])

            def q_phase(half, ns=range(4)):
                c0 = half * 512
                sl = slice(c0, c0 + 512)
                for n in ns:
                    qkvt = qkvp.tile([P, 512], dt.bfloat16, tag="qkvT",
                                     name=f"qkvt{n}")
                    qkv_tile(n, half, qkvt)
                    tc_ = ropep.tile([P, 512], dt.bfloat16, tag="htc", name="htc")
                    nc.vector.tensor_mul(tc_, qkvt, cos_sb[:, sl])
                    ts_ = ropep.tile([P, 512], dt.bfloat16, tag="hts", name="hts")
                    for a in range(0, P, 32):
                        b_ = a ^ 32
                        nc.vector.tensor_mul(ts_[a:a + 32, :],
                                             qkvt[b_:b_ + 32, :],
                                             sin_sb[b_:b_ + 32, sl])
                    for i in range(2):
                        b0 = 64 * i
                        nc.vector.tensor_add(qra[:, 2 * n + i, sl],
                                             tc_[b0:b0 + 64, :],
                                             ts_[b0:b0 + 64, :])

            # ---- attention, split so out_proj(b-1) can fill the exp/mask
            # latency between scores(b) and av(b) on the PE queue ----
            def scores_phase(b):
                pt = ptp.tile([P, 2, HPC, P], dt.bfloat16, tag="pt", name="pt")
                kts = [(0, b - 1), (1, b)] if b > 0 else [(1, b)]
                for g in range(2):
                    g0 = 4 * g
                    for s, kt in kts:
                        ps = pS.tile([P, 4, P], dt.float32, tag="sc", name="sc")
                        nc.tensor.matmul(
                            ps,
                            krope[:, kt * P:(kt + 1) * P],
                            qra[:, g0:g0 + 4, b * P:(b + 1) * P],
                            start=True, stop=True)
                        nc.scalar.activation(pt[:, s, g0:g0 + 4, :], ps,
                                             mybir.ActivationFunctionType.Exp,
                                             bias=zbias, scale=SM_SCALE)
                        m0 = 0 if s == 0 else P
                        nc.vector.tensor_mul(pt[:, s, g0:g0 + 4, :],
                                             pt[:, s, g0:g0 + 4, :],
                                             bcast_mid(mask_sb[:, m0:m0 + P], 4))
                return pt, kts

            def av_phase(b, st):
                pt, kts = st
                rec8 = smallp.tile([P, HPC], dt.float32, tag="rec8", name="rec8")
                an = anp.tile([P, HPC, HD], dt.bfloat16, tag="an", name="an")
                att = attp.tile([P, 4, P], dt.bfloat16, tag="att", name="att")
                a2 = an.rearrange("p a b -> p (a b)")
                for g in range(2):
                    g0 = 4 * g
                    # AV with ones column -> denominators in column 64
                    pg = pG.tile([P, 4, AW], dt.float32, tag="pg", name="pg")
                    for j in range(4):
                        h = g0 + j
                        for idx, (s, kt) in enumerate(kts):
                            nc.tensor.matmul(pg[:, j, :], pt[:, s, h, :],
                                             vtok[:, kt, :],
                                             start=(idx == 0),
                                             stop=(idx == len(kts) - 1))
                    nc.vector.tensor_add(rec8[:, g0:g0 + 4],
                                         pg[:, :, HD:HD + 1],
                                         esink_sb[:, g0:g0 + 4])
                    nc.vector.reciprocal(rec8[:, g0:g0 + 4], rec8[:, g0:g0 + 4])
                    rec3 = bass.AP(tensor=rec8.tensor,
                                   offset=rec8[:, g0:g0 + 4].offset,
                                   ap=[rec8.ap[0], [1, 4], [0, HD]])
                    nc.vector.tensor_mul(an[:, g0:g0 + 4, :], pg[:, :, 0:HD],
                                         rec3)
                # transpose to out-proj lhsT layout [128 hd, 128 tok]
                for kk in range(4):
                    pat = pG.tile([P, P], dt.bfloat16, tag="pg", name="pat")
                    nc.tensor.transpose(pat, a2[:, kk * P:(kk + 1) * P],
                                        identb)
                    if kk % 2 == 0:
                        nc.vector.tensor_copy(att[:, kk, :], pat)
                    else:
                        nc.scalar.copy(att[:, kk, :], pat)
                return att

            # ---- out projection; one y DMA per tile (finer on the last) ----
            def out_proj(b, att):
                ysb = ysbp.tile([P, 6, YC], dt.bfloat16, tag="ysb", name="ysb")
                for ch in range(6):
                    o0 = ch * YC
                    pyt = pY.tile([P, 512], dt.float32, tag="py", name="py")
                    for kk in range(4):
                        nc.tensor.matmul(pyt[:, 0:YC], att[:, kk, :],
                                         wout_sb[kk][:, o0:o0 + YC],
                                         start=(kk == 0), stop=(kk == 3))
                    if ch % 2 == 0:
                        nc.vector.tensor_copy(ysb[:, ch, :], pyt[:, 0:YC])
                    else:
                        nc.scalar.copy(ysb[:, ch, :], pyt[:, 0:YC])
                    if b == MT - 1:
                        for q in range(2):
                            eng = nc.sync if (ch + q) % 2 == 0 else nc.scalar
                            q0 = o0 + q * (YC // 2)
                            eng.dma_start(
                                out=y_d[b * P:(b + 1) * P, q0:q0 + YC // 2],
                                in_=ysb[:, ch, q * (YC // 2):(q + 1) * (YC // 2)])
                if b < MT - 1:
                    (dma if b % 2 == 0 else dma2)(y_d[b * P:(b + 1) * P, :], ysb)

            # ---- schedule ----
            # ssq/rsq for half 1 run after q_phase(0) so nothing on the PE
            # queue waits for the (late) xsq half-1 DMA; all activation-table
            # loads still land before the first attention exp.
            ssq_phase(0)
            rsq_phase(0)
            kv_phase(0)
            q_phase(0)
            ssq_phase(1)
            rsq_phase(1)
            sc = {}
            att_t = {}
            sc[0] = scores_phase(0)
            sc[1] = scores_phase(1)
            att_t[0] = av_phase(0, sc.pop(0))
            sc[2] = scores_phase(2)
            out_proj(0, att_t.pop(0))
            att_t[1] = av_phase(1, sc.pop(1))
            sc[3] = scores_phase(3)
            out_proj(1, att_t.pop(1))
            att_t[2] = av_phase(2, sc.pop(2))
            kv_phase(1)
            out_proj(2, att_t.pop(2))
            att_t[3] = av_phase(3, sc.pop(3))
            q_phase(1)
            sc[4] = scores_phase(4)
            out_proj(3, att_t.pop(3))
            att_t[4] = av_phase(4, sc.pop(4))
            for b in range(5, MT):
                sc[b] = scores_phase(b)
                out_proj(b - 1, att_t.pop(b - 1))
                att_t[b] = av_phase(b, sc.pop(b))
            out_proj(MT - 1, att_t.pop(MT - 1))

    nc.compile()
    return nc


# ----------------------------------------------------------------------------
# public entry
# ----------------------------------------------------------------------------

LAST_RESULTS = None


def kernel(x, norm_scale, qkv_w, qkv_b, out_w, out_b, sinks):
    global LAST_RESULTS
    x = np.asarray(x, dtype=np.float32)
    norm_scale = np.asarray(norm_scale, dtype=np.float32)
    qkv_w = np.asarray(qkv_w, dtype=np.float32)
    qkv_b = np.asarray(qkv_b, dtype=np.float32)
    out_w = np.asarray(out_w, dtype=np.float32)
    out_b = np.asarray(out_b, dtype=np.float32)
    sinks = np.asarray(sinks, dtype=np.float32)

    if "nc" not in _CACHE:
        _CACHE["nc"] = build_nc()
    nc = _CACHE["nc"]

    in_maps = [
        _prep_core_inputs(c, x, norm_scale, qkv_w, qkv_b, out_w, sinks)
        for c in range(NCORES)
    ]
    import os
    tmpdir = os.environ.get("BASS_TMPDIR") or None
    res = run_bass_kernel_spmd(nc, in_maps, core_ids=list(range(NCORES)),
                               tmpdir=tmpdir)
    LAST_RESULTS = res
    y = np.zeros((T, HIDDEN), dtype=np.float64)
    for c in range(NCORES):
        y += res.results[c]["y"].astype(np.float64)
    y += out_b.astype(np.float64)[None, :]
    return y.astype(np.float32)
